# revision 1
# baseline (speedup 1.0000x reference)
"""Trainium2 Bass kernel for nn_AutoregressiveCDF (MADE + rational-quadratic
spline CDF, product over features).

Strategy: pure data-parallel over 8 NeuronCores (batch 16384 -> 8 x 2048),
weights replicated.  Per core:
  Phase A: transpose predicates/contexts via PE, run the MADE trunk as
           hidden-on-partition GEMMs (fp32r on the PE), activations on ACT.
  Phase B: output GEMM t @ W_out in [batch-part, feature-free] orientation,
           PSUM consumed directly by ACT exp/softplus; spline evaluated with a
           monotone-mask formulation (no gather): for each feature the bin
           index idx satisfies u_j = 1[x >= edge_j], and any per-bin quantity
           at idx is a masked sum  sum_j u_j * T_j  (segmented reduce on DVE).
"""

import numpy as np
from contextlib import ExitStack

import concourse.bass as bass
import concourse.bacc as bacc
import concourse.tile as tile
from concourse import mybir
from concourse.bass_utils import run_bass_kernel_spmd

F32 = mybir.dt.float32

# problem sizes (hardcoded per contract)
B, F, H, C = 16384, 64, 512, 512
NB = 30
MULT = 3 * NB + 1            # 91
NBLOCKS = 3
NCORES = 8
MIN_BIN = 1e-3
MIN_DERIV = 1e-3
CFREE = float(1.0 - MIN_BIN * NB)         # softmax mass after min-bin affine
SCALE = float(np.float32(1.0 / np.sqrt(H)))
FH = F // 2                  # features per half (32)
WOH = FH * MULT              # 2912 W_out cols per half
KH = H // 128                # 4 hidden chunks

# knobs (test.py may override module globals before calling kernel())
MM_DT = mybir.dt.float32r    # PE dtype: float32r (fast) or float32 (safe)
USE_SCANMUL = True           # custom DVE scan-mul gathers vs stock mult+reduce
TRACE = False
LAST_RESULTS = None          # BassKernelResults of the most recent run

_CACHE = {}


def _masks():
    d_in = np.arange(1, F + 1)
    d_h = np.arange(H) % max(1, F - 1) + min(1, F - 1)
    m_in = (d_h[None, :] >= d_in[:, None]).astype(np.float32)
    m_hh = (d_h[None, :] >= d_h[:, None]).astype(np.float32)
    d_out = np.repeat(d_in, MULT)
    m_out = (d_out[None, :] > d_h[:, None]).astype(np.float32)
    return m_in, m_hh, m_out


def _scan_mul_ref(in0, in1, s0, s1, imm2):
    a = np.asarray(in0, np.float32).reshape(np.asarray(in0).shape[0], -1)
    b = np.asarray(in1, np.float32).reshape(a.shape)
    return np.cumsum(a * b, axis=1, dtype=np.float32).reshape(
        np.asarray(in0).shape)


def _cumsum_ref(in0, in1, s0, s1, imm2):
    a = np.asarray(in0, np.float32).reshape(np.asarray(in0).shape[0], -1)
    return np.cumsum(a, axis=1, dtype=np.float32).reshape(
        np.asarray(in0).shape)


def _register_scan_mul():
    """Register a fused multiply+prefix-sum DVE op: out = cumsum(in0*in1).

    One DVE pass per masked-sum gather instead of tensor_tensor +
    tensor_reduce; per-feature sums are recovered from segment-boundary
    differences of the chained running sum."""
    import concourse.dve_ops as dve_ops
    from concourse.dve_spec import Spec, Src0, Src1, scan, AluOp, lower
    from concourse.dve_uop import DveOpSpec
    for op in dve_ops.OPS:
        if op.name == "SCAN_MUL_ANT":
            return op
    spec = Spec(body=scan(AluOp.ADD, Src0 * Src1), reference=_scan_mul_ref)
    row = max(dve_ops._SUB_OPCODE_FOR_NAME.values()) + 1
    assert row < 0x20
    shas = {}
    for ver in ("v3", "v4"):
        u = lower(spec, ver=ver)
        shas[ver] = DveOpSpec(name="SCAN_MUL_ANT", opcode=row, uops=u,
                              rd1_en=True).sha(ver)
    op = dve_ops.DveOp("SCAN_MUL_ANT", spec, subdim=False, uops_sha=shas)
    dve_ops.OPS.append(op)
    dve_ops.CUSTOM_DVE_SPECS["SCAN_MUL_ANT"] = spec
    dve_ops._SUB_OPCODE_FOR_NAME["SCAN_MUL_ANT"] = row

    spec2 = Spec(body=scan(AluOp.ADD, Src0), reference=_cumsum_ref)
    row2 = row + 1
    assert row2 < 0x20
    shas2 = {}
    for ver in ("v3", "v4"):
        u2 = lower(spec2, ver=ver)
        shas2[ver] = DveOpSpec(name="CUMSUM_ANT", opcode=row2, uops=u2,
                               rd1_en=False).sha(ver)
    op2 = dve_ops.DveOp("CUMSUM_ANT", spec2, subdim=False, uops_sha=shas2)
    dve_ops.OPS.append(op2)
    dve_ops.CUSTOM_DVE_SPECS["CUMSUM_ANT"] = spec2
    dve_ops._SUB_OPCODE_FOR_NAME["CUMSUM_ANT"] = row2
    return op, op2


class _Bacc(bacc.Bacc):
    """Bacc with a trimmed activation-table list so Exp and Ln share one
    table (no per-chunk ACT_TABLE_LOAD thrash)."""

    _KEEP_TABLES = ("natural_log_exp_and_others", "sigmoid_and_others")

    def insert_act_table_loads(self):
        import bass_rust as _bass_rust
        from concourse.hw_specs import get_activation_tables
        import concourse.mybir as _mb
        has_activation = any(
            isinstance(i, _mb.InstActivation)
            for b in self.main_func.blocks
            for i in b.instructions
        )
        if not has_activation:
            return
        # act_func_set_id is positional in act_info.json order: keep every
        # entry but empty the unwanted ones so the chooser can't pick them.
        all_tables = get_activation_tables(self.m.arch)
        tables = [(k, (v if k in self._KEEP_TABLES else set()))
                  for k, v in all_tables.items()]
        _bass_rust.insert_act_table_loads(self, tables)


def _round_fp32r(a):
    """Round fp32 to the PE's fp32r grid (1s+8e+11m, RNE) on the host."""
    b = np.ascontiguousarray(a, dtype=np.float32).view(np.uint32)
    lsb = (b >> 12) & np.uint32(1)
    b2 = ((b + np.uint32(0x7FF) + lsb) & np.uint32(0xFFFFF000)).astype(np.uint32)
    return b2.view(np.float32)


def _build(bc, mm_dt):
    """Build the per-core Bass module for bc batch rows per core."""
    nch = bc // 128
    MMT = mm_dt
    if USE_SCANMUL:
        scan_mul, cumsum_op = _register_scan_mul()
    else:
        scan_mul = cumsum_op = None
    nc = _Bacc("TRN2", target_bir_lowering=False, debug=False,
               enable_asserts=False)

    def din(name, shape, dt=F32):
        return nc.dram_tensor(name, list(shape), dt, kind="ExternalInput").ap()

    pred = din("pred", (bc, F))
    ctxm = din("ctx", (bc, C))
    w_in = din("w_in", (F, H), MMT)
    wc_in = din("wc_in", (C, H), MMT)
    wb1 = din("wb1", (NBLOCKS, H, H), MMT)
    wb2 = din("wb2", (NBLOCKS, H, H), MMT)
    wcb = din("wcb", (NBLOCKS, C, H), MMT)
    w_out = din("w_out", (H, F * MULT), MMT)
    b1 = din("b1", (H,))
    bb1 = din("bb1", (NBLOCKS, H))
    bb2 = din("bb2", (NBLOCKS, H))
    bcb = din("bcb", (NBLOCKS, H))
    ident = din("ident", (128, 128))
    k1c = din("k1c", (NB - 1,))
    out_d = nc.dram_tensor("out", [bc], F32, kind="ExternalOutput").ap()

    AX = mybir.AxisListType
    OP = mybir.AluOpType
    ACTF = mybir.ActivationFunctionType

    def bcast(ap2d, n):
        """[P, M] AP -> [P, M, n] with stride-0 inner (broadcast along bins)."""
        return bass.AP(tensor=ap2d.tensor, offset=ap2d.offset,
                       ap=list(ap2d.ap) + [[0, n]])

    def pbcast(ap1d, p, n):
        """[n] DRAM AP -> [p, n] with stride-0 partitions (for DMA)."""
        return bass.AP(tensor=ap1d.tensor, offset=ap1d.offset,
                       ap=[[0, p]] + list(ap1d.ap))

    with tile.TileContext(nc) as tc, ExitStack() as ctx:
        const = ctx.enter_context(tc.tile_pool(name="const", bufs=1))
        persist = ctx.enter_context(tc.tile_pool(name="persist", bufs=1))

        ident_t = const.tile([128, 128], F32)
        nc.sync.dma_start(out=ident_t[:], in_=ident)
        k1_t = const.tile([128, NB - 1], F32)
        nc.sync.dma_start(out=k1_t[:], in_=pbcast(k1c, 128, NB - 1))
        one_t = const.tile([128, 1], F32)
        nc.vector.memset(one_t[:], 1.0)
        mb_t = const.tile([128, 1], F32)
        nc.vector.memset(mb_t[:], MIN_BIN)

        # persistent activations
        t_t = [persist.tile([128, bc], MMT, tag=f"t{k}", name=f"t{k}") for k in range(KH)]
        halfprod = persist.tile([128, nch, 2], F32)

        # ---------------- Phase A: transposes + MADE trunk ----------------
        with tc.tile_pool(name="pa", bufs=1) as pa, \
             tc.tile_pool(name="paw", bufs=2) as paw, \
             tc.tile_pool(name="pat", bufs=2) as pat, \
             ExitStack() as actx:

            ctx_T = [pa.tile([128, bc], MMT, tag=f"ctxT{k}", name=f"ctxT{k}") for k in range(KH)]
            x_T = pa.tile([64, bc], MMT)

            w_in_t = pa.tile([64, H], MMT)
            nc.sync.dma_start(out=w_in_t[:], in_=w_in)
            wc_in_t = [pa.tile([128, H], MMT, tag=f"wci{k}", name=f"wci{k}") for k in range(KH)]
            for k in range(KH):
                nc.sync.dma_start(out=wc_in_t[k][:],
                                  in_=wc_in[k * 128:(k + 1) * 128, :])
            b1_t = pa.tile([128, KH], F32)
            nc.sync.dma_start(out=b1_t[:],
                              in_=b1.rearrange("(m p) -> p m", p=128))
            bb1_t = pa.tile([128, NBLOCKS, KH], F32)
            bb2_t = pa.tile([128, NBLOCKS, KH], F32)
            bcb_t = pa.tile([128, NBLOCKS, KH], F32)
            for tt_, src in ((bb1_t, bb1), (bb2_t, bb2), (bcb_t, bcb)):
                nc.sync.dma_start(out=tt_[:],
                                  in_=src.rearrange("i (m p) -> p i m", p=128))

            # transpose ctx and pred chunks on the PE
            pst_cm = tc.tile_pool(name="pst", bufs=2, space="PSUM")
            psa_pool = [None]
            pst = pst_cm.__enter__()
            for c in range(nch):
                ld = pat.tile([128, C], F32, tag="ctxld", name="ctxld")
                nc.sync.dma_start(out=ld[:], in_=ctxm[c * 128:(c + 1) * 128, :])
                for k in range(KH):
                    ps = pst.tile([128, 128], F32, tag="tp", name="tp")
                    nc.tensor.transpose(ps[:], ld[:, k * 128:(k + 1) * 128],
                                        ident_t[:])
                    nc.scalar.activation(out=ctx_T[k][:, c * 128:(c + 1) * 128],
                                         in_=ps[:], func=ACTF.Copy)
                pld = pat.tile([128, F], F32, tag="predld", name="predld")
                nc.sync.dma_start(out=pld[:], in_=pred[c * 128:(c + 1) * 128, :])
                ps = pst.tile([64, 128], F32, tag="tpp", name="tpp")
                nc.tensor.transpose(ps[:], pld[:], ident_t[:])
                nc.scalar.activation(out=x_T[:, c * 128:(c + 1) * 128],
                                     in_=ps[:], func=ACTF.Copy)

            pst_cm.__exit__(None, None, None)
            psa = actx.enter_context(tc.tile_pool(name="psa", bufs=2, space="PSUM"))
            bsw = min(512, bc)
            nbs = bc // bsw
            # input projection: t = x @ (W_in*M) + ctx @ Wc_in + b1
            for bs in range(nbs):
                bsl = slice(bs * bsw, (bs + 1) * bsw)
                for m in range(KH):
                    msl = slice(m * 128, (m + 1) * 128)
                    ps = psa.tile([128, bsw], F32, tag="mm", name="mm")
                    nc.tensor.matmul(ps[:], w_in_t[:, msl],
                                     x_T[:, bsl],
                                     start=True, stop=False)
                    for k in range(KH):
                        nc.tensor.matmul(ps[:], wc_in_t[k][:, msl],
                                         ctx_T[k][:, bsl],
                                         start=False, stop=(k == KH - 1))
                    nc.scalar.activation(out=t_t[m][:, bsl], in_=ps[:],
                                         func=ACTF.Identity, bias=b1_t[:, m:m + 1])

            # residual blocks
            for i in range(NBLOCKS):
                wb1_i = [paw.tile([128, H], MMT, tag=f"wb1_{k}", name=f"wb1_{k}") for k in range(KH)]
                wb2_i = [paw.tile([128, H], MMT, tag=f"wb2_{k}", name=f"wb2_{k}") for k in range(KH)]
                wcb_i = [paw.tile([128, H], MMT, tag=f"wcb_{k}", name=f"wcb_{k}") for k in range(KH)]
                for k in range(KH):
                    ksl = slice(k * 128, (k + 1) * 128)
                    nc.sync.dma_start(out=wb1_i[k][:], in_=wb1[i, ksl, :])
                    nc.sync.dma_start(out=wb2_i[k][:], in_=wb2[i, ksl, :])
                    nc.sync.dma_start(out=wcb_i[k][:], in_=wcb[i, ksl, :])
                for bs in range(nbs):
                    bsl = slice(bs * bsw, (bs + 1) * bsw)
                    h1t = pat.tile([128, KH, bsw], MMT, tag="h1t", name="h1t")
                    for k in range(KH):
                        nc.scalar.activation(out=h1t[:, k, :],
                                             in_=t_t[k][:, bsl],
                                             func=ACTF.Relu)
                    h1 = [h1t[:, k, :] for k in range(KH)]
                    h2t = pat.tile([128, KH, bsw], MMT, tag="h2t", name="h2t")
                    h2 = [h2t[:, k, :] for k in range(KH)]
                    for m in range(KH):
                        msl = slice(m * 128, (m + 1) * 128)
                        ps = psa.tile([128, bsw], F32, tag="mm1", name="mm1")
                        for k in range(KH):
                            nc.tensor.matmul(ps[:], wb1_i[k][:, msl],
                                             h1[k],
                                             start=(k == 0), stop=(k == KH - 1))
                        nc.scalar.activation(out=h2[m], in_=ps[:],
                                             func=ACTF.Relu,
                                             bias=bb1_t[:, i, m:m + 1])
                    for m in range(KH):
                        msl = slice(m * 128, (m + 1) * 128)
                        ps2 = psa.tile([128, bsw], F32, tag="mm2", name="mm2")
                        for k in range(KH):
                            nc.tensor.matmul(ps2[:], wb2_i[k][:, msl],
                                             h2[k],
                                             start=(k == 0), stop=(k == KH - 1))
                        ps3 = psa.tile([128, bsw], F32, tag="mm3", name="mm3")
                        for k in range(KH):
                            nc.tensor.matmul(ps3[:], wcb_i[k][:, msl],
                                             ctx_T[k][:, bsl],
                                             start=(k == 0), stop=(k == KH - 1))
                        g = pat.tile([128, bsw], F32, tag="g", name="g")
                        nc.scalar.activation(out=g[:], in_=ps3[:], func=ACTF.Sigmoid,
                                             bias=bcb_t[:, i, m:m + 1])
                        v = pat.tile([128, bsw], F32, tag="v", name="v")
                        nc.vector.scalar_tensor_tensor(
                            out=v[:], in0=ps2[:], scalar=bb2_t[:, i, m:m + 1],
                            in1=g[:], op0=OP.add, op1=OP.mult)
                        nc.gpsimd.tensor_tensor(out=t_t[m][:, bsl],
                                                 in0=t_t[m][:, bsl], in1=v[:],
                                                 op=OP.add)

        # ---------------- Phase B: output GEMM + spline ----------------
        GRP = min(8, nch)
        assert nch % GRP == 0
        with tc.tile_pool(name="pb", bufs=1) as pb, \
             tc.tile_pool(name="spl", bufs=2) as spl, \
             tc.tile_pool(name="grp", bufs=2) as grp, \
             tc.tile_pool(name="psb", bufs=3, space="PSUM") as psb:

            TS = nc.vector.tensor_scalar
            TT = nc.vector.tensor_tensor

            def tscopy(dst, srcap):
                TS(out=dst, in0=srcap, scalar1=0.0, scalar2=None, op0=OP.add)

            for half in range(2):
                wo_t = [pb.tile([128, WOH], MMT, tag=f"wo{k}", name=f"wo{k}")
                        for k in range(KH)]
                for k in range(KH):
                    nc.sync.dma_start(
                        out=wo_t[k][:],
                        in_=w_out[k * 128:(k + 1) * 128,
                                  half * WOH:(half + 1) * WOH])
                for gidx in range(nch // GRP):
                    def gt(nm):
                        return grp.tile([128, GRP, FH], F32, tag=nm, name=nm)
                    gGL = gt("gGL"); gCR = gt("gCR"); gIDX = gt("gIDX")
                    gSH = gt("gSH"); gX = gt("gX"); gEW0 = gt("gEW0")
                    gEH0 = gt("gEH0"); gD0 = gt("gD0"); gD1 = gt("gD1")
                    gRall = grp.tile([128, GRP, 6, FH], F32, tag="gRall",
                                     name="gRall")

                    for gi in range(GRP):
                        c = gidx * GRP + gi
                        csl = slice(c * 128, (c + 1) * 128)
                        nc.sync.dma_start(
                            out=gX[:, gi, :],
                            in_=pred[csl, half * FH:(half + 1) * FH])
                        EW = spl.tile([128, FH, NB], F32, tag="EW", name="EW")
                        EH = spl.tile([128, FH, NB], F32, tag="EH", name="EH")
                        ED = spl.tile([128, FH, NB + 1], F32, tag="ED", name="ED")
                        for n in range(2):
                            ps = psb.tile([128, 4, 512], F32, tag="pp",
                                          name="pp", bufs=2)
                            for j in range(4):
                                nsl = slice((n * 4 + j) * 364,
                                            (n * 4 + j + 1) * 364)
                                for k in range(KH):
                                    nc.tensor.matmul(
                                        ps[:, j, 0:364],
                                        t_t[k][:, csl],
                                        wo_t[k][:, nsl],
                                        start=(k == 0), stop=(k == KH - 1))
                            psv = bass.AP(tensor=ps[:].tensor,
                                          offset=ps[:].offset,
                                          ap=[ps[:].ap[0], [512, 4], [MULT, 4],
                                              [1, MULT]])
                            fsl = slice(n * 16, (n + 1) * 16)
                            nc.scalar.activation(
                                out=EW[:, fsl, :].rearrange(
                                    "p (a f) n -> p a f n", a=4),
                                in_=psv[:, :, :, 0:NB],
                                func=ACTF.Exp, scale=SCALE)
                            nc.scalar.activation(
                                out=EH[:, fsl, :].rearrange(
                                    "p (a f) n -> p a f n", a=4),
                                in_=psv[:, :, :, NB:2 * NB],
                                func=ACTF.Exp, scale=SCALE)
                            nc.scalar.activation(
                                out=ED[:, fsl, :].rearrange(
                                    "p (a f) n -> p a f n", a=4),
                                in_=psv[:, :, :, 2 * NB:MULT],
                                func=ACTF.Exp)
                        # D = softplus(ud) = ln(exp(ud) + 1), in place over ED
                        D = ED
                        nc.scalar.activation(
                            out=D[:].rearrange("p f n -> p (f n)"),
                            in_=ED[:].rearrange("p f n -> p (f n)"),
                            func=ACTF.Ln, bias=one_t[:])
                        # chained scan of EW across the whole half
                        Gg = spl.tile([128, FH, NB], F32, tag="Gg", name="Gg", bufs=1)
                        if USE_SCANMUL:
                            nc.vector._custom_dve(
                                cumsum_op,
                                out=Gg[:].rearrange("p f n -> p (f n)"),
                                in0=EW[:].rearrange("p f n -> p (f n)"))
                        else:
                            nc.vector.tensor_tensor_scan(
                                out=Gg[:].rearrange("p f n -> p (f n)"),
                                data0=EW[:].rearrange("p f n -> p (f n)"),
                                data1=EW[:].rearrange("p f n -> p (f n)"),
                                initial=0.0, op0=OP.add, op1=OP.bypass)
                        Gl = bass.AP(tensor=Gg[:].tensor,
                                     offset=Gg[:].offset + NB - 1,
                                     ap=[Gg[:].ap[0], [NB, FH]])
                        tscopy(gGL[:, gi, :], Gl)
                        Sw = spl.tile([128, FH], F32, tag="Sw", name="Sw")
                        tscopy(Sw[:, 0:1], Gl[:, 0:1])
                        TT(out=Sw[:, 1:FH], in0=Gl[:, 1:FH],
                           in1=Gl[:, 0:FH - 1], op=OP.subtract)
                        Rw = spl.tile([128, FH], F32, tag="Rw", name="Rw")
                        nc.vector.reciprocal(out=Rw[:], in_=Sw[:])
                        CR = spl.tile([128, FH], F32, tag="CR", name="CR")
                        TS(out=CR[:], in0=Rw[:], scalar1=CFREE, scalar2=None,
                           op0=OP.mult)
                        tscopy(gCR[:, gi, :], CR[:])
                        xp = spl.tile([128, FH], F32, tag="xp", name="xp")
                        tscopy(xp[:, 0:1], gX[:, gi, 0:1])
                        P2 = spl.tile([128, FH], F32, tag="P2", name="P2")
                        TT(out=P2[:, 1:FH], in0=Gl[:, 0:FH - 1],
                           in1=CR[:, 1:FH], op=OP.mult)
                        TT(out=xp[:, 1:FH], in0=gX[:, gi, 1:FH],
                           in1=P2[:, 1:FH], op=OP.add)
                        # masks
                        # XK = x' - K1 is off the critical chain (no Gg dep)
                        XK = spl.tile([128, FH, NB - 1], F32, tag="XK",
                                      name="XK", bufs=1)
                        k1b = bass.AP(tensor=k1_t[:].tensor,
                                      offset=k1_t[:].offset,
                                      ap=[k1_t[:].ap[0], [0, FH], [1, NB - 1]])
                        nc.gpsimd.tensor_tensor(out=XK[:],
                                                in0=bcast(xp[:], NB - 1),
                                                in1=k1b, op=OP.subtract)
                        ENm = spl.tile([128, FH, NB], F32, tag="ENm",
                                       name="ENm", bufs=1)
                        nc.gpsimd.tensor_tensor(out=ENm[:], in0=Gg[:],
                                                in1=bcast(CR[:], NB),
                                                op=OP.mult)
                        u = spl.tile([128, FH, NB - 1], F32, tag="u", name="u")
                        TT(out=u[:], in0=XK[:], in1=ENm[:, :, 0:NB - 1],
                           op=OP.is_ge)
                        nc.vector.tensor_reduce(out=gIDX[:, gi, :], in_=u[:],
                                                axis=AX.X, op=OP.add)
                        nc.vector.tensor_reduce(out=gSH[:, gi, :], in_=EH[:],
                                                axis=AX.X, op=OP.add)
                        dD = spl.tile([128, FH, NB], F32, tag="dD", name="dD", bufs=1)
                        nc.gpsimd.tensor_tensor(out=dD[:],
                                                in0=D[:, :, 1:NB + 1],
                                                in1=D[:, :, 0:NB],
                                                op=OP.subtract)
                        nc.scalar.activation(
                            out=gEW0[:, gi, :],
                            in_=bass.AP(tensor=EW[:].tensor,
                                        offset=EW[:].offset,
                                        ap=[EW[:].ap[0], [NB, FH]]),
                            func=ACTF.Copy)
                        nc.scalar.activation(
                            out=gEH0[:, gi, :],
                            in_=bass.AP(tensor=EH[:].tensor,
                                        offset=EH[:].offset,
                                        ap=[EH[:].ap[0], [NB, FH]]),
                            func=ACTF.Copy)
                        nc.scalar.activation(
                            out=gD0[:, gi, :],
                            in_=bass.AP(tensor=D[:].tensor, offset=D[:].offset,
                                        ap=[D[:].ap[0], [NB + 1, FH]]),
                            func=ACTF.Copy)
                        nc.scalar.activation(
                            out=gD1[:, gi, :],
                            in_=bass.AP(tensor=D[:].tensor,
                                        offset=D[:].offset + 1,
                                        ap=[D[:].ap[0], [NB + 1, FH]]),
                            func=ACTF.Copy)
                        streams = (EW[:, :, 0:NB - 1], EW[:, :, 1:NB],
                                   EH[:, :, 0:NB - 1], EH[:, :, 1:NB],
                                   dD[:, :, 0:NB - 1], dD[:, :, 1:NB])
                        Rbig = spl.tile([128, 6, FH, NB - 1], F32,
                                        tag="Rbig", name="Rbig", bufs=1)
                        for i_s, tsl in enumerate(streams):
                            if USE_SCANMUL:
                                nc.vector._custom_dve(scan_mul,
                                                      out=Rbig[:, i_s, :, :],
                                                      in0=u[:], in1=tsl)
                            else:
                                TT(out=Rbig[:, i_s, :, :], in0=u[:], in1=tsl,
                                   op=OP.mult)
                                nc.vector.tensor_reduce(
                                    out=gRall[:, gi, i_s, :],
                                    in_=Rbig[:, i_s, :, :],
                                    axis=AX.X, op=OP.add)
                        if USE_SCANMUL:
                            # one extraction for all six gathers
                            Rl6 = bass.AP(tensor=Rbig[:].tensor,
                                          offset=Rbig[:].offset + NB - 2,
                                          ap=[Rbig[:].ap[0], [FH * (NB - 1), 6],
                                              [NB - 1, FH]])
                            tscopy(gRall[:, gi, :, :], Rl6)

                    # ---- grouped small chain on [128, GRP, FH] tiles ----
                    def g2t(nm):
                        return grp.tile([128, GRP, FH], F32, tag=nm, name=nm,
                                        bufs=1)
                    if USE_SCANMUL:
                        gdall = grp.tile([128, GRP, 6, FH], F32, tag="gdall",
                                         name="gdall", bufs=1)
                        TT(out=gdall[:, :, :, 1:FH],
                           in0=gRall[:, :, :, 1:FH],
                           in1=gRall[:, :, :, 0:FH - 1], op=OP.subtract)
                        tscopy(gdall[:, :, :, 0:1], gRall[:, :, :, 0:1])
                    else:
                        gdall = gRall
                    g1 = gdall[:, :, 0, :]
                    g2_ = gdall[:, :, 1, :]
                    g3 = gdall[:, :, 2, :]
                    g4 = gdall[:, :, 3, :]
                    g5 = gdall[:, :, 4, :]
                    g6 = gdall[:, :, 5, :]
                    t1 = g2t("t1")
                    nc.scalar.activation(out=t1[:], in_=gIDX[:],
                                         func=ACTF.Copy, scale=MIN_BIN)
                    incw = g2t("incw")
                    TT(out=incw[:], in0=gCR[:], in1=g1, op=OP.mult)
                    TT(out=incw[:], in0=incw[:], in1=t1[:], op=OP.add)
                    ewi = g2t("ewi")
                    TT(out=ewi[:], in0=g2_, in1=g1, op=OP.subtract)
                    TT(out=ewi[:], in0=ewi[:], in1=gEW0[:], op=OP.add)
                    inw = g2t("inw")
                    TT(out=inw[:], in0=gCR[:], in1=ewi[:], op=OP.mult)
                    nc.scalar.activation(out=inw[:], in_=inw[:],
                                         func=ACTF.Identity, bias=mb_t[:])
                    rw_ = g2t("rw_")
                    nc.vector.reciprocal(out=rw_[:], in_=inw[:])
                    th = g2t("th")
                    TT(out=th[:], in0=gX[:], in1=incw[:], op=OP.subtract)
                    TT(out=th[:], in0=th[:], in1=rw_[:], op=OP.mult)
                    gRH = g2t("gRH")
                    nc.vector.reciprocal(out=gRH[:], in_=gSH[:])
                    gCH = g2t("gCH")
                    TS(out=gCH[:], in0=gRH[:], scalar1=CFREE, scalar2=None,
                       op0=OP.mult)
                    inch = g2t("inch")
                    TT(out=inch[:], in0=gCH[:], in1=g3, op=OP.mult)
                    TT(out=inch[:], in0=inch[:], in1=t1[:], op=OP.add)
                    ehi = g2t("ehi")
                    TT(out=ehi[:], in0=g4, in1=g3, op=OP.subtract)
                    TT(out=ehi[:], in0=ehi[:], in1=gEH0[:], op=OP.add)
                    inh = g2t("inh")
                    TT(out=inh[:], in0=gCH[:], in1=ehi[:], op=OP.mult)
                    nc.scalar.activation(out=inh[:], in_=inh[:],
                                         func=ACTF.Identity, bias=mb_t[:])
                    ind = g2t("ind")
                    nc.vector.scalar_tensor_tensor(out=ind[:], in0=g5,
                                                   scalar=MIN_DERIV,
                                                   in1=gD0[:], op0=OP.add,
                                                   op1=OP.add)
                    indp = g2t("indp")
                    nc.vector.scalar_tensor_tensor(out=indp[:], in0=g6,
                                                   scalar=MIN_DERIV,
                                                   in1=gD1[:], op0=OP.add,
                                                   op1=OP.add)
                    dl = g2t("dl")
                    TT(out=dl[:], in0=inh[:], in1=rw_[:], op=OP.mult)
                    om = g2t("om")
                    nc.scalar.activation(out=om[:], in_=th[:],
                                         func=ACTF.Identity, bias=one_t[:],
                                         scale=-1.0)
                    ttv = g2t("ttv")
                    TT(out=ttv[:], in0=th[:], in1=om[:], op=OP.mult)
                    th2 = g2t("th2")
                    nc.scalar.activation(out=th2[:], in_=th[:],
                                         func=ACTF.Square)
                    na = g2t("na")
                    TT(out=na[:], in0=dl[:], in1=th2[:], op=OP.mult)
                    nb_ = g2t("nb_")
                    TT(out=nb_[:], in0=ind[:], in1=ttv[:], op=OP.mult)
                    TT(out=na[:], in0=na[:], in1=nb_[:], op=OP.add)
                    TT(out=na[:], in0=na[:], in1=inh[:], op=OP.mult)
                    s1_ = g2t("s1_")
                    TT(out=s1_[:], in0=ind[:], in1=indp[:], op=OP.add)
                    nc.vector.scalar_tensor_tensor(out=s1_[:], in0=dl[:],
                                                   scalar=-2.0, in1=s1_[:],
                                                   op0=OP.mult, op1=OP.add)
                    TT(out=s1_[:], in0=s1_[:], in1=ttv[:], op=OP.mult)
                    TT(out=s1_[:], in0=s1_[:], in1=dl[:], op=OP.add)
                    rden = g2t("rden")
                    nc.vector.reciprocal(out=rden[:], in_=s1_[:])
                    cdf = g2t("cdf")
                    TT(out=cdf[:], in0=na[:], in1=rden[:], op=OP.mult)
                    TT(out=cdf[:], in0=cdf[:], in1=inch[:], op=OP.add)
                    # product over the 32 features of this half
                    TT(out=cdf[:, :, 0:16], in0=cdf[:, :, 0:16],
                       in1=cdf[:, :, 16:32], op=OP.mult)
                    TT(out=cdf[:, :, 0:8], in0=cdf[:, :, 0:8],
                       in1=cdf[:, :, 8:16], op=OP.mult)
                    TT(out=cdf[:, :, 0:4], in0=cdf[:, :, 0:4],
                       in1=cdf[:, :, 4:8], op=OP.mult)
                    TT(out=cdf[:, :, 0:2], in0=cdf[:, :, 0:2],
                       in1=cdf[:, :, 2:4], op=OP.mult)
                    TT(out=halfprod[:, gidx * GRP:(gidx + 1) * GRP,
                                    half:half + 1],
                       in0=cdf[:, :, 0:1], in1=cdf[:, :, 1:2], op=OP.mult)

            fp = persist.tile([128, nch], F32)
            nc.vector.tensor_tensor(
                out=fp[:],
                in0=halfprod[:, :, 0:1].rearrange("p c h -> p (c h)"),
                in1=halfprod[:, :, 1:2].rearrange("p c h -> p (c h)"),
                op=OP.mult)
            nc.sync.dma_start(out=out_d.rearrange("(c p) -> p c", p=128),
                              in_=fp[:])

    nc.compile()
    return nc


def _prep_shared(W_in, b_in, Wc_in, bc_in, Wb1, bb1, Wb2, bb2, Wcb, bcb,
                 W_out, b_out, mm_dt):
    m_in, m_hh, m_out = _masks()
    assert not np.any(b_out), "nonzero b_out not supported by this kernel"
    rnd = _round_fp32r if mm_dt == mybir.dt.float32r else (
        lambda a: np.ascontiguousarray(a, dtype=np.float32))
    shared = {
        "w_in": rnd(W_in * m_in),
        "wc_in": rnd(Wc_in),
        "wb1": rnd(Wb1 * m_hh[None]),
        "wb2": rnd(Wb2 * m_hh[None]),
        "wcb": rnd(Wcb),
        "w_out": rnd(W_out * m_out),
        "b1": np.ascontiguousarray((b_in + bc_in).astype(np.float32)),
        "bb1": np.ascontiguousarray(bb1.astype(np.float32)),
        "bb2": np.ascontiguousarray(bb2.astype(np.float32)),
        "bcb": np.ascontiguousarray(bcb.astype(np.float32)),
        "ident": np.eye(128, dtype=np.float32),
        "k1c": (MIN_BIN * np.arange(1, NB)).astype(np.float32),
    }
    return shared


def kernel(predicates, contexts, W_in, b_in, Wc_in, bc_in, Wb1, bb1, Wb2, bb2,
           Wcb, bcb, W_out, b_out):
    global LAST_RESULTS
    predicates = np.asarray(predicates, dtype=np.float32)
    contexts = np.asarray(contexts, dtype=np.float32)
    bc = predicates.shape[0] // NCORES
    key = (bc, MM_DT, USE_SCANMUL)
    if key not in _CACHE:
        _CACHE[key] = _build(bc, MM_DT)
    nc = _CACHE[key]
    shared = _prep_shared(W_in, b_in, Wc_in, bc_in, Wb1, bb1, Wb2, bb2,
                          Wcb, bcb, W_out, b_out, MM_DT)
    in_maps = []
    for cid in range(NCORES):
        sl = slice(cid * bc, (cid + 1) * bc)
        m = dict(shared)
        m["pred"] = np.ascontiguousarray(predicates[sl])
        m["ctx"] = np.ascontiguousarray(contexts[sl])
        in_maps.append(m)
    res = run_bass_kernel_spmd(nc, in_maps, core_ids=list(range(NCORES)),
                               trace=TRACE)
    LAST_RESULTS = res
    return np.concatenate([res.results[i]["out"] for i in range(NCORES)])



# revision 17
# speedup vs baseline: 1.0521x; 1.0521x over previous
"""Trainium2 Bass kernel for nn_AutoregressiveCDF (MADE + rational-quadratic
spline CDF, product over features).  v2.

Strategy: pure data-parallel over 8 NeuronCores (batch 16384 -> 8 x 2048).
Per core, row-block pipelined (4 blocks x 512 rows):

- MADE hidden units are degree-sorted offline, which makes the masked H x H
  weights block-upper-triangular at 128 granularity: mm1/mm2 skip 6/16 chunk
  matmuls, and the output GEMM contracts only K = q+1 hidden chunks for
  feature quarter q (10/16 of the chunk-matmuls).
- All GEMMs in bf16 (weights and moving operands); PSUM fp32.
- The 4 context projections (input + 3 GLU gates) are fused into one
  stationary [C, 4H] and computed per block.
- Spline per 128-row chunk: exp/softplus on ACT straight out of PSUM (W_out
  columns are pre-reordered so uw|uh|ud land contiguously per feature
  quarter); cumsum + masked-sum scans on DVE with custom ops whose OUTPUT AP
  has a stride-0 inner dim, so each scan writes only its per-feature running
  total ("capture") - no full-width masked prefix arrays, no extraction
  passes.  Edges/compares/reductions on GpSimd.  The per-feature rational
  quadratic chain is deferred to block granularity on [128, 4*64] tiles.
"""

import numpy as np
from contextlib import ExitStack

import concourse.bass as bass
import concourse.bacc as bacc
import concourse.tile as tile
from concourse import mybir
from concourse.bass_utils import run_bass_kernel_spmd

F32 = mybir.dt.float32
BF16 = mybir.dt.bfloat16

# problem sizes (hardcoded per contract)
B, F, H, C = 16384, 64, 512, 512
NB = 30
MULT = 3 * NB + 1            # 91
NBLOCKS = 3
NCORES = 8
MIN_BIN = 1e-3
MIN_DERIV = 1e-3
CF = float(1.0 - MIN_BIN * NB)
SCALE = float(np.float32(1.0 / np.sqrt(H)))
KH = H // 128                 # 4 hidden chunks
NQ = 4                        # feature quarters
FQ = F // NQ                  # 16 features per quarter
WQ = FQ * NB                  # 480 w/h cols per quarter
DQ = FQ * (NB + 1)            # 496 d cols per quarter
QCOLS = 2 * WQ + DQ           # 1456 cols per quarter
PAIRS = ((0, 3), (1, 2))      # psum pairing: both pairs need 5 k-matmuls

# knobs
TRACE = False
MM_DT = BF16                  # kept for test.py compat
GP_REDUCE = False             # gpsimd tensor_reduce can't do free-axis; DVE
DVE_BCAST = False             # stride-0 bcast in0 on DVE TT broken on HW
LAST_RESULTS = None
DBG = None                    # debug-dump tensor name (see _build dbg_shapes)

_CACHE = {}


def _masks():
    d_in = np.arange(1, F + 1)
    d_h = np.arange(H) % max(1, F - 1) + min(1, F - 1)
    m_in = (d_h[None, :] >= d_in[:, None]).astype(np.float32)
    m_hh = (d_h[None, :] >= d_h[:, None]).astype(np.float32)
    d_out = np.repeat(d_in, MULT)
    m_out = (d_out[None, :] > d_h[:, None]).astype(np.float32)
    return m_in, m_hh, m_out, d_h


def _scan_mul_ref(in0, in1, s0, s1, imm2):
    a = np.asarray(in0, np.float32).reshape(np.asarray(in0).shape[0], -1)
    b = np.asarray(in1, np.float32).reshape(a.shape)
    return np.cumsum(a * b, axis=1, dtype=np.float32).reshape(
        np.asarray(in0).shape)


def _cumsum_ref(in0, in1, s0, s1, imm2):
    a = np.asarray(in0, np.float32).reshape(np.asarray(in0).shape[0], -1)
    return np.cumsum(a, axis=1, dtype=np.float32).reshape(
        np.asarray(in0).shape)


def _register_scan_mul():
    """Fused multiply+prefix-sum DVE op (state fp32): out = cumsum(in0*in1),
    plus a plain cumsum. Used with stride-0 output APs as segmented 'capture'
    reductions."""
    import concourse.dve_ops as dve_ops
    from concourse.dve_spec import Spec, Src0, Src1, scan, AluOp, lower
    from concourse.dve_uop import DveOpSpec
    have = {op.name: op for op in dve_ops.OPS}
    if "SCAN_MUL_ANT" in have and "CUMSUM_ANT" in have:
        return have["SCAN_MUL_ANT"], have["CUMSUM_ANT"]
    spec = Spec(body=scan(AluOp.ADD, Src0 * Src1), reference=_scan_mul_ref)
    row = max(dve_ops._SUB_OPCODE_FOR_NAME.values()) + 1
    assert row + 1 < 0x20
    shas = {}
    for ver in ("v3", "v4"):
        u = lower(spec, ver=ver)
        shas[ver] = DveOpSpec(name="SCAN_MUL_ANT", opcode=row, uops=u,
                              rd1_en=True).sha(ver)
    op = dve_ops.DveOp("SCAN_MUL_ANT", spec, subdim=False, uops_sha=shas)
    dve_ops.OPS.append(op)
    dve_ops.CUSTOM_DVE_SPECS["SCAN_MUL_ANT"] = spec
    dve_ops._SUB_OPCODE_FOR_NAME["SCAN_MUL_ANT"] = row

    spec2 = Spec(body=scan(AluOp.ADD, Src0), reference=_cumsum_ref)
    row2 = row + 1
    shas2 = {}
    for ver in ("v3", "v4"):
        u2 = lower(spec2, ver=ver)
        shas2[ver] = DveOpSpec(name="CUMSUM_ANT", opcode=row2, uops=u2,
                               rd1_en=False).sha(ver)
    op2 = dve_ops.DveOp("CUMSUM_ANT", spec2, subdim=False, uops_sha=shas2)
    dve_ops.OPS.append(op2)
    dve_ops.CUSTOM_DVE_SPECS["CUMSUM_ANT"] = spec2
    dve_ops._SUB_OPCODE_FOR_NAME["CUMSUM_ANT"] = row2
    return op, op2


class _Bacc(bacc.Bacc):
    """Bacc with a trimmed activation-table list so Exp and Ln share one
    table and Sigmoid another (no per-chunk ACT_TABLE_LOAD thrash)."""

    _KEEP_TABLES = ("natural_log_exp_and_others", "sigmoid_and_others")

    def insert_act_table_loads(self):
        import bass_rust as _bass_rust
        from concourse.hw_specs import get_activation_tables
        import concourse.mybir as _mb
        has_activation = any(
            isinstance(i, _mb.InstActivation)
            for b in self.main_func.blocks
            for i in b.instructions
        )
        if not has_activation:
            return
        all_tables = get_activation_tables(self.m.arch)
        tables = [(k, (v if k in self._KEEP_TABLES else set()))
                  for k, v in all_tables.items()]
        _bass_rust.insert_act_table_loads(self, tables)


def _build(bc, dbg=None):
    nch = bc // 128               # 16 row chunks of 128
    NBLK = 4
    BSW = bc // NBLK              # 512 rows per block
    GPB = BSW // 128              # 4 chunks per block
    scan_mul, cumsum_op = _register_scan_mul()
    nc = _Bacc("TRN2", target_bir_lowering=False, debug=False,
               enable_asserts=False)

    def din(name, shape, dt=F32):
        return nc.dram_tensor(name, list(shape), dt, kind="ExternalInput").ap()

    dbg_shapes = {
        "tbf": ([NBLK, 128, KH, bc // NBLK], BF16),
        "EW": ([bc // 128, 128, F, NB], BF16),
        "EH": ([bc // 128, 128, F, NB], BF16),
        "D": ([bc // 128, 128, F, NB + 1], BF16),
        "Gg": ([bc // 128, 128, F, NB], F32),
        "u": ([bc // 128, 128, F, NB - 1], BF16),
        "cap": ([NBLK, 128, bc // NBLK // 128, 6, F], F32),
        "sm": ([NBLK, 128, 3, bc // NBLK // 128, F], F32),
        "ENk": ([bc // 128, 128, F, NB - 1], F32),
        "xpc": ([bc // 128, 128, F], F32),
    }
    dbg_d = None
    if dbg is not None:
        shp, ddt = dbg_shapes[dbg]
        dbg_d = nc.dram_tensor("dbg", list(shp), ddt,
                               kind="ExternalOutput").ap()

    pred = din("pred", (bc, F))
    ctxm = din("ctx", (bc, C))
    win = din("win", (F, H), BF16)
    wc4 = din("wc4", (C, (NBLOCKS + 1) * H), BF16)
    wb1 = din("wb1", (NBLOCKS, H, H), BF16)
    wb2 = din("wb2", (NBLOCKS, H, H), BF16)
    wo_d = [din(f"wo{k}", (128, (NQ - k) * QCOLS), BF16) for k in range(KH)]
    b1 = din("b1", (H,))
    bb1 = din("bb1", (NBLOCKS, H))
    bb2 = din("bb2", (NBLOCKS, H))
    bcb = din("bcb", (NBLOCKS, H))
    ident = din("ident", (128, 128))
    k1c = din("k1c", (NB - 1,))
    out_d = nc.dram_tensor("out", [bc], F32, kind="ExternalOutput").ap()

    AX = mybir.AxisListType
    OP = mybir.AluOpType
    ACTF = mybir.ActivationFunctionType

    def bcast(ap2d, n):
        """[P, M] AP -> [P, M, n] stride-0 inner."""
        return bass.AP(tensor=ap2d.tensor, offset=ap2d.offset,
                       ap=list(ap2d.ap) + [[0, n]])

    def pbcast(ap1d, p, n):
        return bass.AP(tensor=ap1d.tensor, offset=ap1d.offset,
                       ap=[[0, p]] + list(ap1d.ap))

    with tile.TileContext(nc) as tc, ExitStack() as ctx:
        const = ctx.enter_context(tc.tile_pool(name="const", bufs=1))
        wp = ctx.enter_context(tc.tile_pool(name="wp", bufs=1))
        persist = ctx.enter_context(tc.tile_pool(name="persist", bufs=1))

        ident_t = const.tile([128, 128], F32)
        nc.sync.dma_start(out=ident_t[:], in_=ident)
        one_t = const.tile([128, 1], F32)
        nc.vector.memset(one_t[:], 1.0)
        k1_t = const.tile([128, NB - 1], F32)
        nc.sync.dma_start(out=k1_t[:], in_=pbcast(k1c, 128, NB - 1))

        # ---- weights ----
        win_t = wp.tile([F, H], BF16)
        nc.sync.dma_start(out=win_t[:], in_=win)
        wc4_t = [wp.tile([128, (NBLOCKS + 1) * H], BF16, tag=f"wc4_{k}",
                         name=f"wc4_{k}") for k in range(KH)]
        for k in range(KH):
            nc.sync.dma_start(out=wc4_t[k][:],
                              in_=wc4[k * 128:(k + 1) * 128, :])
        wb1_t = [[wp.tile([128, H - 128 * k], BF16, tag=f"wb1_{i}_{k}",
                          name=f"wb1_{i}_{k}") for k in range(KH)]
                 for i in range(NBLOCKS)]
        wb2_t = [[wp.tile([128, H - 128 * k], BF16, tag=f"wb2_{i}_{k}",
                          name=f"wb2_{i}_{k}") for k in range(KH)]
                 for i in range(NBLOCKS)]
        for i in range(NBLOCKS):
            for k in range(KH):
                ksl = slice(k * 128, (k + 1) * 128)
                nc.sync.dma_start(out=wb1_t[i][k][:],
                                  in_=wb1[i, ksl, 128 * k:])
                nc.sync.dma_start(out=wb2_t[i][k][:],
                                  in_=wb2[i, ksl, 128 * k:])
        wo_t = [wp.tile([128, (NQ - k) * QCOLS], BF16, tag=f"wo_{k}",
                        name=f"wo_{k}") for k in range(KH)]
        for k in range(KH):
            nc.sync.dma_start(out=wo_t[k][:], in_=wo_d[k])
        b1_t = wp.tile([128, KH], F32)
        nc.sync.dma_start(out=b1_t[:], in_=b1.rearrange("(m p) -> p m", p=128))
        bb1_t = wp.tile([128, NBLOCKS, KH], F32)
        bb2_t = wp.tile([128, NBLOCKS, KH], F32)
        bcb_t = wp.tile([128, NBLOCKS, KH], F32)
        for tt_, src in ((bb1_t, bb1), (bb2_t, bb2), (bcb_t, bcb)):
            nc.sync.dma_start(out=tt_[:],
                              in_=src.rearrange("i (m p) -> p i m", p=128))

        # ---- persistent activations ----
        ctx_T = [persist.tile([128, bc], BF16, tag=f"ctxT{k}", name=f"ctxT{k}")
                 for k in range(KH)]
        x_T = persist.tile([F, bc], BF16)
        prodb = persist.tile([128, nch], F32)

        # ---- transposes (prologue) ----
        with tc.tile_pool(name="pst", bufs=2, space="PSUM") as pst, \
             tc.tile_pool(name="pld", bufs=2) as pld:
            for c in range(nch):
                ld = pld.tile([128, C], F32, tag="ctxld", name="ctxld")
                nc.sync.dma_start(out=ld[:], in_=ctxm[c * 128:(c + 1) * 128, :])
                for k in range(KH):
                    ps = pst.tile([128, 128], F32, tag="tp", name="tp")
                    nc.tensor.transpose(ps[:], ld[:, k * 128:(k + 1) * 128],
                                        ident_t[:])
                    nc.scalar.activation(out=ctx_T[k][:, c * 128:(c + 1) * 128],
                                         in_=ps[:], func=ACTF.Copy)
                pldp = pld.tile([128, F], F32, tag="predld", name="predld")
                nc.sync.dma_start(out=pldp[:],
                                  in_=pred[c * 128:(c + 1) * 128, :])
                ps = pst.tile([F, 128], F32, tag="tpp", name="tpp")
                nc.tensor.transpose(ps[:], pldp[:], ident_t[:])
                nc.scalar.activation(out=x_T[:, c * 128:(c + 1) * 128],
                                     in_=ps[:], func=ACTF.Copy)

        TS = nc.vector.tensor_scalar
        TT = nc.vector.tensor_tensor
        STT = nc.vector.scalar_tensor_tensor

        def tscopy(dst, srcap):
            TS(out=dst, in0=srcap, scalar1=0.0, scalar2=None, op0=OP.add)

        # ---- main pipeline pools ----
        with tc.tile_pool(name="psa", bufs=2, space="PSUM") as psa, \
             tc.tile_pool(name="psb", bufs=3, space="PSUM") as psb, \
             tc.tile_pool(name="blk", bufs=2) as blkp, \
             tc.tile_pool(name="blk1", bufs=1) as blkp1, \
             tc.tile_pool(name="spl", bufs=2) as spl, \
             tc.tile_pool(name="spl1", bufs=1) as spl1, \
             tc.tile_pool(name="win", bufs=1) as winp, \
             tc.tile_pool(name="chn", bufs=1) as chn:

            for blk in range(NBLK):
                bsl = slice(blk * BSW, (blk + 1) * BSW)

                # ---- t0 + gates ----
                t_t = blkp.tile([128, KH, BSW], F32, tag="t", name="t")
                for m in range(KH):
                    msl = slice(m * 128, (m + 1) * 128)
                    ps = psa.tile([128, BSW], F32, tag="mma", name="mma")
                    nc.tensor.matmul(ps[:], win_t[:, msl], x_T[:, bsl],
                                     start=True, stop=False)
                    for k in range(KH):
                        nc.tensor.matmul(ps[:], wc4_t[k][:, msl],
                                         ctx_T[k][:, bsl],
                                         start=False, stop=(k == KH - 1))
                    nc.scalar.activation(out=t_t[:, m, :], in_=ps[:],
                                         func=ACTF.Identity,
                                         bias=b1_t[:, m:m + 1])
                g_t = blkp1.tile([128, NBLOCKS, KH, BSW], BF16, tag="g",
                                 name="g")
                for i in range(NBLOCKS):
                    for m in range(KH):
                        csl0 = (i + 1) * H + m * 128
                        ps = psa.tile([128, BSW], F32, tag="mma", name="mma")
                        for k in range(KH):
                            nc.tensor.matmul(ps[:],
                                             wc4_t[k][:, csl0:csl0 + 128],
                                             ctx_T[k][:, bsl],
                                             start=(k == 0), stop=(k == KH - 1))
                        nc.scalar.activation(out=g_t[:, i, m, :], in_=ps[:],
                                             func=ACTF.Sigmoid,
                                             bias=bcb_t[:, i, m:m + 1])

                # ---- residual trunk (block-sparse) ----
                for i in range(NBLOCKS):
                    h1 = blkp1.tile([128, KH, BSW], BF16, tag="h1", name="h1")
                    for m in range(KH):
                        nc.scalar.activation(out=h1[:, m, :], in_=t_t[:, m, :],
                                             func=ACTF.Relu)
                    h2 = blkp1.tile([128, KH, BSW], BF16, tag="h2", name="h2")
                    for m in range(KH):
                        ps = psa.tile([128, BSW], F32, tag="mma", name="mma")
                        for k in range(m + 1):
                            off = (m - k) * 128
                            nc.tensor.matmul(ps[:],
                                             wb1_t[i][k][:, off:off + 128],
                                             h1[:, k, :],
                                             start=(k == 0), stop=(k == m))
                        nc.scalar.activation(out=h2[:, m, :], in_=ps[:],
                                             func=ACTF.Relu,
                                             bias=bb1_t[:, i, m:m + 1])
                    for m in range(KH):
                        ps2 = psa.tile([128, BSW], F32, tag="mma", name="mma")
                        for k in range(m + 1):
                            off = (m - k) * 128
                            nc.tensor.matmul(ps2[:],
                                             wb2_t[i][k][:, off:off + 128],
                                             h2[:, k, :],
                                             start=(k == 0), stop=(k == m))
                        v = blkp1.tile([128, BSW], F32, tag="v", name="v")
                        STT(out=v[:], in0=ps2[:], scalar=bb2_t[:, i, m:m + 1],
                            in1=g_t[:, i, m, :], op0=OP.add, op1=OP.mult)
                        nc.gpsimd.tensor_tensor(out=t_t[:, m, :],
                                                in0=t_t[:, m, :], in1=v[:],
                                                op=OP.add)
                tbf = blkp1.tile([128, KH, BSW], BF16, tag="tbf", name="tbf")
                for m in range(KH):
                    tscopy(tbf[:, m, :], t_t[:, m, :])
                if dbg == "tbf":
                    nc.sync.dma_start(out=dbg_d[blk], in_=tbf[:])

                # ---- per-block window buffers ----
                capb = winp.tile([128, GPB, 6, F], F32, tag="capb", name="capb")
                extb = winp.tile([128, GPB, 4, F], F32, tag="extb", name="extb")
                xb = winp.tile([128, GPB, F], F32, tag="xb", name="xb")
                crb = winp.tile([128, GPB, F], F32, tag="crb", name="crb")
                idxb = winp.tile([128, GPB, F], F32, tag="idxb", name="idxb")
                shb = winp.tile([128, GPB, F], F32, tag="shb", name="shb")

                # ---- out-GEMM + spline per 128-row chunk ----
                for gi in range(GPB):
                    c = blk * GPB + gi
                    csl = slice(c * 128, (c + 1) * 128)
                    gsl = slice(gi * 128, (gi + 1) * 128)
                    nc.sync.dma_start(out=xb[:, gi, :], in_=pred[csl, :])

                    EW = spl.tile([128, F, NB], BF16, tag="EW", name="EW")
                    EH = spl.tile([128, F, NB], BF16, tag="EH", name="EH")
                    Dt = spl1.tile([128, F, NB + 1], BF16, tag="Dt", name="Dt")
                    for ty in range(3):       # 0=w 1=h 2=d
                        for pa, pair in enumerate(PAIRS):
                            ncols = DQ if ty == 2 else WQ
                            ps = psb.tile([128, 2, 512], F32, tag="po",
                                          name="po")
                            for si, q in enumerate(pair):
                                for k in range(q + 1):
                                    off = ((q - k) * QCOLS + ty * WQ)
                                    nc.tensor.matmul(
                                        ps[:, si, 0:ncols],
                                        tbf[:, k, gsl],
                                        wo_t[k][:, off:off + ncols],
                                        start=(k == 0), stop=(k == q))
                            for si, q in enumerate(pair):
                                fsl = slice(q * FQ, (q + 1) * FQ)
                                if ty == 0:
                                    nc.scalar.activation(
                                        out=EW[:, fsl, :], in_=ps[:, si, 0:ncols],
                                        func=ACTF.Exp, scale=SCALE)
                                elif ty == 1:
                                    nc.scalar.activation(
                                        out=EH[:, fsl, :], in_=ps[:, si, 0:ncols],
                                        func=ACTF.Exp, scale=SCALE)
                                else:
                                    nc.scalar.activation(
                                        out=Dt[:, fsl, :], in_=ps[:, si, 0:ncols],
                                        func=ACTF.Exp)
                    # D = ln(exp(ud) + 1) in place
                    nc.scalar.activation(
                        out=Dt[:].rearrange("p f n -> p (f n)"),
                        in_=Dt[:].rearrange("p f n -> p (f n)"),
                        func=ACTF.Ln, bias=one_t[:])
                    dD = spl.tile([128, F, NB], BF16, tag="dD", name="dD")
                    nc.gpsimd.tensor_tensor(out=dD[:], in0=Dt[:, :, 1:NB + 1],
                                            in1=Dt[:, :, 0:NB], op=OP.subtract)
                    # extracts: EW0, EH0, D0, D1
                    for j, (src, st, o) in enumerate((
                            (EW, NB, 0), (EH, NB, 0), (Dt, NB + 1, 0),
                            (Dt, NB + 1, 1))):
                        nc.scalar.activation(
                            out=extb[:, gi, j, :],
                            in_=bass.AP(tensor=src[:].tensor,
                                        offset=src[:].offset + o,
                                        ap=[src[:].ap[0], [st, F]]),
                            func=ACTF.Copy)

                    if dbg == "EW":
                        nc.sync.dma_start(out=dbg_d[c], in_=EW[:])
                    if dbg == "EH":
                        nc.sync.dma_start(out=dbg_d[c], in_=EH[:])
                    if dbg == "D":
                        nc.sync.dma_start(out=dbg_d[c], in_=Dt[:])
                    # chained cumsum of EW (fp32) for edges
                    Gg = spl1.tile([128, F, NB], F32, tag="Gg", name="Gg")
                    nc.vector._custom_dve(
                        cumsum_op,
                        out=Gg[:].rearrange("p f n -> p (f n)"),
                        in0=EW[:].rearrange("p f n -> p (f n)"))
                    # per-feature smalls: Gl, S, rS, CR, xpc
                    Gl = spl1.tile([128, F], F32, tag="Gl", name="Gl")
                    tscopy(Gl[:], bass.AP(tensor=Gg[:].tensor,
                                          offset=Gg[:].offset + NB - 1,
                                          ap=[Gg[:].ap[0], [NB, F]]))
                    Sf = spl1.tile([128, F], F32, tag="Sf", name="Sf")
                    tscopy(Sf[:, 0:1], Gl[:, 0:1])
                    TT(out=Sf[:, 1:F], in0=Gl[:, 1:F], in1=Gl[:, 0:F - 1],
                       op=OP.subtract)
                    rS = spl1.tile([128, F], F32, tag="rS", name="rS")
                    nc.vector.reciprocal_approx_fast(out=rS[:], in_=Sf[:])
                    TS(out=crb[:, gi, :], in0=rS[:], scalar1=CF, scalar2=None,
                       op0=OP.mult)
                    glcr = spl1.tile([128, F], F32, tag="glcr", name="glcr")
                    # glcr[f] = Gl[f-1] * CR[f]  (current feature's CR!)
                    TT(out=glcr[:, 1:F], in0=Gl[:, 0:F - 1],
                       in1=crb[:, gi, 1:F], op=OP.mult)
                    xpc = spl1.tile([128, F], F32, tag="xpc", name="xpc")
                    tscopy(xpc[:, 0:1], xb[:, gi, 0:1])
                    TT(out=xpc[:, 1:F], in0=xb[:, gi, 1:F],
                       in1=glcr[:, 1:F], op=OP.add)

                    # edges: ENk = Gg[:, :, 0:29] * CR + k1
                    ENk = spl1.tile([128, F, NB - 1], F32, tag="ENk",
                                    name="ENk")
                    nc.gpsimd.tensor_tensor(out=ENk[:], in0=Gg[:, :, 0:NB - 1],
                                            in1=bcast(crb[:, gi, :], NB - 1),
                                            op=OP.mult)
                    k1b = bass.AP(tensor=k1_t[:].tensor, offset=k1_t[:].offset,
                                  ap=[k1_t[:].ap[0], [0, F], [1, NB - 1]])
                    nc.gpsimd.tensor_tensor(out=ENk[:], in0=ENk[:], in1=k1b,
                                            op=OP.add)
                    # u = xpc >= ENk
                    u = spl.tile([128, F, NB - 1], BF16, tag="u", name="u")
                    if DVE_BCAST:
                        TT(out=u[:], in0=bcast(xpc[:], NB - 1), in1=ENk[:],
                           op=OP.is_ge)
                    else:
                        xkb = spl1.tile([128, F, NB - 1], F32, tag="xkb",
                                        name="xkb")
                        nc.gpsimd.tensor_tensor(out=xkb[:],
                                                in0=bcast(xpc[:], NB - 1),
                                                in1=ENk[:], op=OP.subtract)
                        TS(out=u[:], in0=xkb[:], scalar1=0.0, scalar2=None,
                           op0=OP.is_ge)
                    if dbg == "Gg":
                        nc.sync.dma_start(out=dbg_d[c], in_=Gg[:])
                    if dbg == "ENk":
                        nc.sync.dma_start(out=dbg_d[c], in_=ENk[:])
                    if dbg == "xpc":
                        nc.sync.dma_start(out=dbg_d[c], in_=xpc[:])
                    if dbg == "u":
                        nc.sync.dma_start(out=dbg_d[c], in_=u[:])
                    # idx and SH reductions
                    if GP_REDUCE:
                        nc.gpsimd.tensor_reduce(out=idxb[:, gi, :], in_=u[:],
                                                axis=AX.X, op=OP.add)
                        nc.gpsimd.tensor_reduce(out=shb[:, gi, :], in_=EH[:],
                                                axis=AX.X, op=OP.add)
                    else:
                        nc.vector.tensor_reduce(out=idxb[:, gi, :], in_=u[:],
                                                axis=AX.X, op=OP.add)
                        nc.vector.tensor_reduce(out=shb[:, gi, :], in_=EH[:],
                                                axis=AX.X, op=OP.add)

                    # six masked-sum scans with stride-0 capture outputs
                    streams = (EW[:, :, 0:NB - 1], EW[:, :, 1:NB],
                               EH[:, :, 0:NB - 1], EH[:, :, 1:NB],
                               dD[:, :, 0:NB - 1], dD[:, :, 1:NB])
                    for s, tsl in enumerate(streams):
                        cap = bass.AP(
                            tensor=capb[:].tensor,
                            offset=capb[:].offset + (gi * 6 + s) * F,
                            ap=[capb[:].ap[0], [1, F], [0, NB - 1]])
                        nc.vector._custom_dve(scan_mul, out=cap,
                                              in0=u[:], in1=tsl)

                if dbg == "cap":
                    nc.sync.dma_start(out=dbg_d[blk], in_=capb[:])
                if dbg == "sm":
                    nc.sync.dma_start(out=dbg_d[blk, :, 0], in_=idxb[:])
                    nc.sync.dma_start(out=dbg_d[blk, :, 1], in_=shb[:])
                    nc.sync.dma_start(out=dbg_d[blk, :, 2], in_=crb[:])
                # ---- deferred per-feature chain, [128, GPB*F] tiles ----
                def ct(nm):
                    return chn.tile([128, GPB, F], F32, tag=nm, name=nm)

                def flat(t):
                    return t[:].rearrange("p a f -> p (a f)")

                # un-chain captures: gd[f] = cap[f] - cap[f-1] within each gi
                gd = chn.tile([128, GPB, 6, F], F32, tag="gd", name="gd")
                tscopy(gd[:, :, :, 0:1], capb[:, :, :, 0:1])
                TT(out=gd[:, :, :, 1:F], in0=capb[:, :, :, 1:F],
                   in1=capb[:, :, :, 0:F - 1], op=OP.subtract)
                g1 = gd[:, :, 0, :]
                g2 = gd[:, :, 1, :]
                g3 = gd[:, :, 2, :]
                g4 = gd[:, :, 3, :]
                g5 = gd[:, :, 4, :]
                g6 = gd[:, :, 5, :]
                ew0 = extb[:, :, 0, :]
                eh0 = extb[:, :, 1, :]
                d0e = extb[:, :, 2, :]
                d1e = extb[:, :, 3, :]

                w1 = ct("w1"); w2 = ct("w2"); w3 = ct("w3")
                w6 = ct("w6"); w7 = ct("w7"); w8 = ct("w8"); w9 = ct("w9")
                # w1 = incw = g1*CR + mb*idx
                TT(out=w1[:], in0=g1, in1=crb[:], op=OP.mult)
                STT(out=w1[:], in0=idxb[:], scalar=MIN_BIN, in1=w1[:],
                    op0=OP.mult, op1=OP.add)
                # w2 = ewi -> inw -> rw
                TT(out=w2[:], in0=g2, in1=g1, op=OP.subtract)
                nc.gpsimd.tensor_tensor(out=w2[:], in0=w2[:], in1=ew0,
                                        op=OP.add)
                TT(out=w2[:], in0=w2[:], in1=crb[:], op=OP.mult)
                TS(out=w2[:], in0=w2[:], scalar1=MIN_BIN, scalar2=None,
                   op0=OP.add)
                nc.vector.reciprocal_approx_fast(out=flat(w2), in_=flat(w2))
                # w3 = th
                TT(out=w3[:], in0=xb[:], in1=w1[:], op=OP.subtract)
                TT(out=w3[:], in0=w3[:], in1=w2[:], op=OP.mult)
                # w1 = ch2 ; shb <- inch ; w6 = inh
                nc.vector.reciprocal_approx_fast(out=flat(w1), in_=flat(shb))
                TS(out=w1[:], in0=w1[:], scalar1=CF, scalar2=None, op0=OP.mult)
                TT(out=w6[:], in0=g4, in1=g3, op=OP.subtract)
                nc.gpsimd.tensor_tensor(out=w6[:], in0=w6[:], in1=eh0,
                                        op=OP.add)
                TT(out=w6[:], in0=w6[:], in1=w1[:], op=OP.mult)
                TS(out=w6[:], in0=w6[:], scalar1=MIN_BIN, scalar2=None,
                   op0=OP.add)
                TT(out=shb[:], in0=g3, in1=w1[:], op=OP.mult)
                STT(out=shb[:], in0=idxb[:], scalar=MIN_BIN, in1=shb[:],
                    op0=OP.mult, op1=OP.add)
                # w1 = ind ; w7 = ttv ; w8 = th2
                STT(out=w1[:], in0=g5, scalar=MIN_DERIV, in1=d0e,
                    op0=OP.add, op1=OP.add)
                nc.scalar.activation(out=flat(w7), in_=flat(w3),
                                     func=ACTF.Identity, bias=one_t[:],
                                     scale=-1.0)
                nc.gpsimd.tensor_tensor(out=w7[:], in0=w3[:], in1=w7[:],
                                        op=OP.mult)
                nc.scalar.activation(out=flat(w8), in_=flat(w3),
                                     func=ACTF.Square)
                # w9 = dl
                TT(out=w9[:], in0=w6[:], in1=w2[:], op=OP.mult)
                # w3 = num (th dead)
                TT(out=w3[:], in0=w9[:], in1=w8[:], op=OP.mult)
                nc.gpsimd.tensor_tensor(out=w8[:], in0=w1[:], in1=w7[:],
                                        op=OP.mult)
                TT(out=w3[:], in0=w3[:], in1=w8[:], op=OP.add)
                TT(out=w3[:], in0=w3[:], in1=w6[:], op=OP.mult)
                # w8 = den -> rden
                STT(out=w8[:], in0=g6, scalar=MIN_DERIV, in1=d1e,
                    op0=OP.add, op1=OP.add)
                TT(out=w8[:], in0=w8[:], in1=w1[:], op=OP.add)
                STT(out=w8[:], in0=w9[:], scalar=-2.0, in1=w8[:],
                    op0=OP.mult, op1=OP.add)
                TT(out=w8[:], in0=w8[:], in1=w7[:], op=OP.mult)
                TT(out=w8[:], in0=w8[:], in1=w9[:], op=OP.add)
                nc.vector.reciprocal_approx_fast(out=flat(w8), in_=flat(w8))
                cdf = w3
                TT(out=cdf[:], in0=cdf[:], in1=w8[:], op=OP.mult)
                TT(out=cdf[:], in0=cdf[:], in1=shb[:], op=OP.add)
                # product over 64 features
                TT(out=cdf[:, :, 0:32], in0=cdf[:, :, 0:32],
                   in1=cdf[:, :, 32:64], op=OP.mult)
                TT(out=cdf[:, :, 0:16], in0=cdf[:, :, 0:16],
                   in1=cdf[:, :, 16:32], op=OP.mult)
                TT(out=cdf[:, :, 0:8], in0=cdf[:, :, 0:8],
                   in1=cdf[:, :, 8:16], op=OP.mult)
                TT(out=cdf[:, :, 0:4], in0=cdf[:, :, 0:4],
                   in1=cdf[:, :, 4:8], op=OP.mult)
                TT(out=cdf[:, :, 0:2], in0=cdf[:, :, 0:2],
                   in1=cdf[:, :, 2:4], op=OP.mult)
                TT(out=prodb[:, blk * GPB:(blk + 1) * GPB],
                   in0=cdf[:, :, 0:1].rearrange("p a f -> p (a f)"),
                   in1=cdf[:, :, 1:2].rearrange("p a f -> p (a f)"),
                   op=OP.mult)

            nc.sync.dma_start(out=out_d.rearrange("(c p) -> p c", p=128),
                              in_=prodb[:])

    nc.compile()
    return nc


def _to_bf16(a):
    import ml_dtypes
    return np.ascontiguousarray(np.asarray(a, np.float32)).astype(
        ml_dtypes.bfloat16)


def _prep_shared(W_in, b_in, Wc_in, bc_in, Wb1, bb1, Wb2, bb2, Wcb, bcb,
                 W_out, b_out):
    m_in, m_hh, m_out, d_h = _masks()
    assert not np.any(np.asarray(b_out)), "nonzero b_out not supported"
    perm = np.argsort(d_h, kind="stable")

    W_in = np.asarray(W_in, np.float32) * m_in
    W_in = W_in[:, perm]
    Wc_in = np.asarray(Wc_in, np.float32)[:, perm]
    Wcb = np.asarray(Wcb, np.float32)[:, :, perm]
    Wb1p = (np.asarray(Wb1, np.float32) * m_hh[None])[:, perm][:, :, perm]
    Wb2p = (np.asarray(Wb2, np.float32) * m_hh[None])[:, perm][:, :, perm]
    W_outp = (np.asarray(W_out, np.float32) * m_out)[perm, :]
    b1 = (np.asarray(b_in, np.float32) + np.asarray(bc_in, np.float32))[perm]
    bb1p = np.asarray(bb1, np.float32)[:, perm]
    bb2p = np.asarray(bb2, np.float32)[:, perm]
    bcbp = np.asarray(bcb, np.float32)[:, perm]

    wc4 = np.concatenate([Wc_in, Wcb[0], Wcb[1], Wcb[2]], axis=1)

    # packed W_out blobs: chunk k holds quarters q>=k as [w(480)|h(480)|d(496)]
    wo = []
    for k in range(KH):
        cols = []
        for q in range(k, NQ):
            for ty in range(3):
                for f in range(FQ * q, FQ * (q + 1)):
                    if ty == 0:
                        cols.extend(range(f * MULT, f * MULT + NB))
                    elif ty == 1:
                        cols.extend(range(f * MULT + NB, f * MULT + 2 * NB))
                    else:
                        cols.extend(range(f * MULT + 2 * NB, (f + 1) * MULT))
        blob = W_outp[k * 128:(k + 1) * 128][:, cols]
        assert blob.shape == (128, (NQ - k) * QCOLS)
        wo.append(_to_bf16(blob))

    shared = {
        "win": _to_bf16(W_in),
        "wc4": _to_bf16(wc4),
        "wb1": _to_bf16(Wb1p),
        "wb2": _to_bf16(Wb2p),
        "b1": np.ascontiguousarray(b1),
        "bb1": np.ascontiguousarray(bb1p),
        "bb2": np.ascontiguousarray(bb2p),
        "bcb": np.ascontiguousarray(bcbp),
        "ident": np.eye(128, dtype=np.float32),
        "k1c": (MIN_BIN * np.arange(1, NB)).astype(np.float32),
    }
    for k in range(KH):
        shared[f"wo{k}"] = wo[k]
    return shared


def kernel(predicates, contexts, W_in, b_in, Wc_in, bc_in, Wb1, bb1, Wb2, bb2,
           Wcb, bcb, W_out, b_out):
    global LAST_RESULTS
    predicates = np.asarray(predicates, dtype=np.float32)
    contexts = np.asarray(contexts, dtype=np.float32)
    bc = predicates.shape[0] // NCORES
    key = (bc, GP_REDUCE, DVE_BCAST, DBG)
    if key not in _CACHE:
        _CACHE[key] = _build(bc, dbg=DBG)
    nc = _CACHE[key]
    shared = _prep_shared(W_in, b_in, Wc_in, bc_in, Wb1, bb1, Wb2, bb2,
                          Wcb, bcb, W_out, b_out)
    in_maps = []
    for cid in range(NCORES):
        sl = slice(cid * bc, (cid + 1) * bc)
        m = dict(shared)
        m["pred"] = np.ascontiguousarray(predicates[sl])
        m["ctx"] = np.ascontiguousarray(contexts[sl])
        in_maps.append(m)
    res = run_bass_kernel_spmd(nc, in_maps, core_ids=list(range(NCORES)),
                               trace=TRACE)
    LAST_RESULTS = res
    return np.concatenate([res.results[i]["out"] for i in range(NCORES)])


# revision 18
# speedup vs baseline: 1.2793x; 1.2159x over previous
"""Trainium2 Bass kernel for nn_AutoregressiveCDF (MADE + rational-quadratic
spline CDF, product over features).  v2.

Strategy: pure data-parallel over 8 NeuronCores (batch 16384 -> 8 x 2048).
Per core, row-block pipelined (4 blocks x 512 rows):

- MADE hidden units are degree-sorted offline, which makes the masked H x H
  weights block-upper-triangular at 128 granularity: mm1/mm2 skip 6/16 chunk
  matmuls, and the output GEMM contracts only K = q+1 hidden chunks for
  feature quarter q (10/16 of the chunk-matmuls).
- All GEMMs in bf16 (weights and moving operands); PSUM fp32.
- The 4 context projections (input + 3 GLU gates) are fused into one
  stationary [C, 4H] and computed per block.
- Spline per 128-row chunk: exp/softplus on ACT straight out of PSUM (W_out
  columns are pre-reordered so uw|uh|ud land contiguously per feature
  quarter); cumsum + masked-sum scans on DVE with custom ops whose OUTPUT AP
  has a stride-0 inner dim, so each scan writes only its per-feature running
  total ("capture") - no full-width masked prefix arrays, no extraction
  passes.  Edges/compares/reductions on GpSimd.  The per-feature rational
  quadratic chain is deferred to block granularity on [128, 4*64] tiles.
"""

import numpy as np
from contextlib import ExitStack

import concourse.bass as bass
import concourse.bacc as bacc
import concourse.tile as tile
from concourse import mybir
from concourse.bass_utils import run_bass_kernel_spmd

F32 = mybir.dt.float32
BF16 = mybir.dt.bfloat16

# problem sizes (hardcoded per contract)
B, F, H, C = 16384, 64, 512, 512
NB = 30
MULT = 3 * NB + 1            # 91
NBLOCKS = 3
NCORES = 8
MIN_BIN = 1e-3
MIN_DERIV = 1e-3
CF = float(1.0 - MIN_BIN * NB)
SCALE = float(np.float32(1.0 / np.sqrt(H)))
KH = H // 128                 # 4 hidden chunks
NQ = 4                        # feature quarters
FQ = F // NQ                  # 16 features per quarter
WQ = FQ * NB                  # 480 w/h cols per quarter
DQ = FQ * (NB + 1)            # 496 d cols per quarter
QCOLS = 2 * WQ + DQ           # 1456 cols per quarter
PAIRS = ((0, 3), (1, 2))      # psum pairing: both pairs need 5 k-matmuls

# knobs
TRACE = False
MM_DT = BF16                  # kept for test.py compat
GP_REDUCE = False             # gpsimd tensor_reduce can't do free-axis; DVE
DVE_BCAST = True              # stride-0 bcast in0 on DVE TT works
LAST_RESULTS = None
DBG = None                    # debug-dump tensor name (see _build dbg_shapes)

_CACHE = {}


def _masks():
    d_in = np.arange(1, F + 1)
    d_h = np.arange(H) % max(1, F - 1) + min(1, F - 1)
    m_in = (d_h[None, :] >= d_in[:, None]).astype(np.float32)
    m_hh = (d_h[None, :] >= d_h[:, None]).astype(np.float32)
    d_out = np.repeat(d_in, MULT)
    m_out = (d_out[None, :] > d_h[:, None]).astype(np.float32)
    return m_in, m_hh, m_out, d_h


def _scan_mul_ref(in0, in1, s0, s1, imm2):
    a = np.asarray(in0, np.float32).reshape(np.asarray(in0).shape[0], -1)
    b = np.asarray(in1, np.float32).reshape(a.shape)
    return np.cumsum(a * b, axis=1, dtype=np.float32).reshape(
        np.asarray(in0).shape)


def _cumsum_ref(in0, in1, s0, s1, imm2):
    a = np.asarray(in0, np.float32).reshape(np.asarray(in0).shape[0], -1)
    return np.cumsum(a, axis=1, dtype=np.float32).reshape(
        np.asarray(in0).shape)


def _register_scan_mul():
    """Fused multiply+prefix-sum DVE op (state fp32): out = cumsum(in0*in1),
    plus a plain cumsum. Used with stride-0 output APs as segmented 'capture'
    reductions."""
    import concourse.dve_ops as dve_ops
    from concourse.dve_spec import Spec, Src0, Src1, scan, AluOp, lower
    from concourse.dve_uop import DveOpSpec
    have = {op.name: op for op in dve_ops.OPS}
    if "SCAN_MUL_ANT" in have and "CUMSUM_ANT" in have:
        return have["SCAN_MUL_ANT"], have["CUMSUM_ANT"]
    spec = Spec(body=scan(AluOp.ADD, Src0 * Src1), reference=_scan_mul_ref)
    row = max(dve_ops._SUB_OPCODE_FOR_NAME.values()) + 1
    assert row + 1 < 0x20
    shas = {}
    for ver in ("v3", "v4"):
        u = lower(spec, ver=ver)
        shas[ver] = DveOpSpec(name="SCAN_MUL_ANT", opcode=row, uops=u,
                              rd1_en=True).sha(ver)
    op = dve_ops.DveOp("SCAN_MUL_ANT", spec, subdim=False, uops_sha=shas)
    dve_ops.OPS.append(op)
    dve_ops.CUSTOM_DVE_SPECS["SCAN_MUL_ANT"] = spec
    dve_ops._SUB_OPCODE_FOR_NAME["SCAN_MUL_ANT"] = row

    spec2 = Spec(body=scan(AluOp.ADD, Src0), reference=_cumsum_ref)
    row2 = row + 1
    shas2 = {}
    for ver in ("v3", "v4"):
        u2 = lower(spec2, ver=ver)
        shas2[ver] = DveOpSpec(name="CUMSUM_ANT", opcode=row2, uops=u2,
                               rd1_en=False).sha(ver)
    op2 = dve_ops.DveOp("CUMSUM_ANT", spec2, subdim=False, uops_sha=shas2)
    dve_ops.OPS.append(op2)
    dve_ops.CUSTOM_DVE_SPECS["CUMSUM_ANT"] = spec2
    dve_ops._SUB_OPCODE_FOR_NAME["CUMSUM_ANT"] = row2
    return op, op2


class _Bacc(bacc.Bacc):
    """Bacc with a trimmed activation-table list so Exp and Ln share one
    table and Sigmoid another (no per-chunk ACT_TABLE_LOAD thrash)."""

    _KEEP_TABLES = ("natural_log_exp_and_others", "sigmoid_and_others")

    def insert_act_table_loads(self):
        import bass_rust as _bass_rust
        from concourse.hw_specs import get_activation_tables
        import concourse.mybir as _mb
        has_activation = any(
            isinstance(i, _mb.InstActivation)
            for b in self.main_func.blocks
            for i in b.instructions
        )
        if not has_activation:
            return
        all_tables = get_activation_tables(self.m.arch)
        tables = [(k, (v if k in self._KEEP_TABLES else set()))
                  for k, v in all_tables.items()]
        _bass_rust.insert_act_table_loads(self, tables)


def _build(bc, dbg=None):
    nch = bc // 128               # 16 row chunks of 128
    NBLK = 4
    BSW = bc // NBLK              # 512 rows per block
    GPB = BSW // 128              # 4 chunks per block
    scan_mul, cumsum_op = _register_scan_mul()
    nc = _Bacc("TRN2", target_bir_lowering=False, debug=False,
               enable_asserts=False)

    def din(name, shape, dt=F32):
        return nc.dram_tensor(name, list(shape), dt, kind="ExternalInput").ap()

    dbg_shapes = {
        "tbf": ([NBLK, 128, KH, bc // NBLK], BF16),
        "EW": ([bc // 128, 128, F, NB], BF16),
        "EH": ([bc // 128, 128, F, NB], BF16),
        "D": ([bc // 128, 128, F, NB + 1], BF16),
        "Gg": ([bc // 128, 128, F, NB], F32),
        "u": ([bc // 128, 128, F, NB - 1], BF16),
        "cap": ([NBLK, 128, bc // NBLK // 128, 6, F], F32),
        "sm": ([NBLK, 128, 3, bc // NBLK // 128, F], F32),
        "ENk": ([bc // 128, 128, F, NB - 1], F32),
        "xpc": ([bc // 128, 128, F], F32),
    }
    dbg_d = None
    if dbg is not None:
        shp, ddt = dbg_shapes[dbg]
        dbg_d = nc.dram_tensor("dbg", list(shp), ddt,
                               kind="ExternalOutput").ap()

    pred = din("pred", (bc, F))
    ctxm = din("ctx", (bc, C))
    win = din("win", (F, H), BF16)
    wc4 = din("wc4", (C, (NBLOCKS + 1) * H), BF16)
    wb1 = din("wb1", (NBLOCKS, H, H), BF16)
    wb2 = din("wb2", (NBLOCKS, H, H), BF16)
    wo_d = [din(f"wo{k}", (128, (NQ - k) * QCOLS), BF16) for k in range(KH)]
    b1 = din("b1", (H,))
    bb1 = din("bb1", (NBLOCKS, H))
    bb2 = din("bb2", (NBLOCKS, H))
    bcb = din("bcb", (NBLOCKS, H))
    ident = din("ident", (128, 128))
    k1c = din("k1c", (NB - 1,))
    out_d = nc.dram_tensor("out", [bc], F32, kind="ExternalOutput").ap()

    AX = mybir.AxisListType
    OP = mybir.AluOpType
    ACTF = mybir.ActivationFunctionType

    def bcast(ap2d, n):
        """[P, M] AP -> [P, M, n] stride-0 inner."""
        return bass.AP(tensor=ap2d.tensor, offset=ap2d.offset,
                       ap=list(ap2d.ap) + [[0, n]])

    def pbcast(ap1d, p, n):
        return bass.AP(tensor=ap1d.tensor, offset=ap1d.offset,
                       ap=[[0, p]] + list(ap1d.ap))

    with tile.TileContext(nc) as tc, ExitStack() as ctx:
        const = ctx.enter_context(tc.tile_pool(name="const", bufs=1))
        wp = ctx.enter_context(tc.tile_pool(name="wp", bufs=1))
        persist = ctx.enter_context(tc.tile_pool(name="persist", bufs=1))

        ident_t = const.tile([128, 128], F32)
        nc.sync.dma_start(out=ident_t[:], in_=ident)
        one_t = const.tile([128, 1], F32)
        nc.vector.memset(one_t[:], 1.0)
        k1_t = const.tile([128, NB - 1], F32)
        nc.sync.dma_start(out=k1_t[:], in_=pbcast(k1c, 128, NB - 1))

        # ---- weights ----
        win_t = wp.tile([F, H], BF16)
        nc.sync.dma_start(out=win_t[:], in_=win)
        wc4_t = [wp.tile([128, (NBLOCKS + 1) * H], BF16, tag=f"wc4_{k}",
                         name=f"wc4_{k}") for k in range(KH)]
        for k in range(KH):
            nc.sync.dma_start(out=wc4_t[k][:],
                              in_=wc4[k * 128:(k + 1) * 128, :])
        wb1_t = [[wp.tile([128, H - 128 * k], BF16, tag=f"wb1_{i}_{k}",
                          name=f"wb1_{i}_{k}") for k in range(KH)]
                 for i in range(NBLOCKS)]
        wb2_t = [[wp.tile([128, H - 128 * k], BF16, tag=f"wb2_{i}_{k}",
                          name=f"wb2_{i}_{k}") for k in range(KH)]
                 for i in range(NBLOCKS)]
        for i in range(NBLOCKS):
            for k in range(KH):
                ksl = slice(k * 128, (k + 1) * 128)
                nc.sync.dma_start(out=wb1_t[i][k][:],
                                  in_=wb1[i, ksl, 128 * k:])
                nc.sync.dma_start(out=wb2_t[i][k][:],
                                  in_=wb2[i, ksl, 128 * k:])
        wo_t = [wp.tile([128, (NQ - k) * QCOLS], BF16, tag=f"wo_{k}",
                        name=f"wo_{k}") for k in range(KH)]
        for k in range(KH):
            nc.sync.dma_start(out=wo_t[k][:], in_=wo_d[k])
        b1_t = wp.tile([128, KH], F32)
        nc.sync.dma_start(out=b1_t[:], in_=b1.rearrange("(m p) -> p m", p=128))
        bb1_t = wp.tile([128, NBLOCKS, KH], F32)
        bb2_t = wp.tile([128, NBLOCKS, KH], F32)
        bcb_t = wp.tile([128, NBLOCKS, KH], F32)
        for tt_, src in ((bb1_t, bb1), (bb2_t, bb2), (bcb_t, bcb)):
            nc.sync.dma_start(out=tt_[:],
                              in_=src.rearrange("i (m p) -> p i m", p=128))

        # ---- persistent activations ----
        ctx_T = [persist.tile([128, bc], BF16, tag=f"ctxT{k}", name=f"ctxT{k}")
                 for k in range(KH)]
        x_T = persist.tile([F, bc], BF16)
        prodb = persist.tile([128, nch], F32)

        # ---- transposes (prologue) ----
        with tc.tile_pool(name="pst", bufs=2, space="PSUM") as pst, \
             tc.tile_pool(name="pld", bufs=2) as pld:
            for c in range(nch):
                ld = pld.tile([128, C], F32, tag="ctxld", name="ctxld")
                nc.sync.dma_start(out=ld[:], in_=ctxm[c * 128:(c + 1) * 128, :])
                for k in range(KH):
                    ps = pst.tile([128, 128], F32, tag="tp", name="tp")
                    nc.tensor.transpose(ps[:], ld[:, k * 128:(k + 1) * 128],
                                        ident_t[:])
                    nc.scalar.activation(out=ctx_T[k][:, c * 128:(c + 1) * 128],
                                         in_=ps[:], func=ACTF.Copy)
                pldp = pld.tile([128, F], F32, tag="predld", name="predld")
                nc.sync.dma_start(out=pldp[:],
                                  in_=pred[c * 128:(c + 1) * 128, :])
                ps = pst.tile([F, 128], F32, tag="tpp", name="tpp")
                nc.tensor.transpose(ps[:], pldp[:], ident_t[:])
                nc.scalar.activation(out=x_T[:, c * 128:(c + 1) * 128],
                                     in_=ps[:], func=ACTF.Copy)

        TS = nc.vector.tensor_scalar
        TT = nc.vector.tensor_tensor
        STT = nc.vector.scalar_tensor_tensor

        def tscopy(dst, srcap):
            TS(out=dst, in0=srcap, scalar1=0.0, scalar2=None, op0=OP.add)

        # ---- main pipeline pools ----
        with tc.tile_pool(name="psa", bufs=2, space="PSUM") as psa, \
             tc.tile_pool(name="psb", bufs=3, space="PSUM") as psb, \
             tc.tile_pool(name="blk", bufs=2) as blkp, \
             tc.tile_pool(name="blk1", bufs=1) as blkp1, \
             tc.tile_pool(name="spl", bufs=2) as spl, \
             tc.tile_pool(name="spl1", bufs=1) as spl1, \
             tc.tile_pool(name="win", bufs=1) as winp, \
             tc.tile_pool(name="chn", bufs=1) as chn:

            for blk in range(NBLK):
                bsl = slice(blk * BSW, (blk + 1) * BSW)

                # ---- t0 + gates ----
                t_t = blkp.tile([128, KH, BSW], F32, tag="t", name="t")
                for m in range(KH):
                    msl = slice(m * 128, (m + 1) * 128)
                    ps = psa.tile([128, BSW], F32, tag="mma", name="mma")
                    nc.tensor.matmul(ps[:], win_t[:, msl], x_T[:, bsl],
                                     start=True, stop=False)
                    for k in range(KH):
                        nc.tensor.matmul(ps[:], wc4_t[k][:, msl],
                                         ctx_T[k][:, bsl],
                                         start=False, stop=(k == KH - 1))
                    nc.scalar.activation(out=t_t[:, m, :], in_=ps[:],
                                         func=ACTF.Identity,
                                         bias=b1_t[:, m:m + 1])
                g_t = blkp1.tile([128, NBLOCKS, KH, BSW], BF16, tag="g",
                                 name="g")
                for i in range(NBLOCKS):
                    for m in range(KH):
                        csl0 = (i + 1) * H + m * 128
                        ps = psa.tile([128, BSW], F32, tag="mma", name="mma")
                        for k in range(KH):
                            nc.tensor.matmul(ps[:],
                                             wc4_t[k][:, csl0:csl0 + 128],
                                             ctx_T[k][:, bsl],
                                             start=(k == 0), stop=(k == KH - 1))
                        nc.scalar.activation(out=g_t[:, i, m, :], in_=ps[:],
                                             func=ACTF.Sigmoid,
                                             bias=bcb_t[:, i, m:m + 1])

                # ---- residual trunk (block-sparse) ----
                for i in range(NBLOCKS):
                    h1 = blkp1.tile([128, KH, BSW], BF16, tag="h1", name="h1")
                    for m in range(KH):
                        nc.scalar.activation(out=h1[:, m, :], in_=t_t[:, m, :],
                                             func=ACTF.Relu)
                    h2 = blkp1.tile([128, KH, BSW], BF16, tag="h2", name="h2")
                    for m in range(KH):
                        ps = psa.tile([128, BSW], F32, tag="mma", name="mma")
                        for k in range(m + 1):
                            off = (m - k) * 128
                            nc.tensor.matmul(ps[:],
                                             wb1_t[i][k][:, off:off + 128],
                                             h1[:, k, :],
                                             start=(k == 0), stop=(k == m))
                        nc.scalar.activation(out=h2[:, m, :], in_=ps[:],
                                             func=ACTF.Relu,
                                             bias=bb1_t[:, i, m:m + 1])
                    for m in range(KH):
                        ps2 = psa.tile([128, BSW], F32, tag="mma", name="mma")
                        for k in range(m + 1):
                            off = (m - k) * 128
                            nc.tensor.matmul(ps2[:],
                                             wb2_t[i][k][:, off:off + 128],
                                             h2[:, k, :],
                                             start=(k == 0), stop=(k == m))
                        v = blkp1.tile([128, BSW], F32, tag="v", name="v")
                        STT(out=v[:], in0=ps2[:], scalar=bb2_t[:, i, m:m + 1],
                            in1=g_t[:, i, m, :], op0=OP.add, op1=OP.mult)
                        nc.gpsimd.tensor_tensor(out=t_t[:, m, :],
                                                in0=t_t[:, m, :], in1=v[:],
                                                op=OP.add)
                tbf = blkp1.tile([128, KH, BSW], BF16, tag="tbf", name="tbf")
                for m in range(KH):
                    tscopy(tbf[:, m, :], t_t[:, m, :])
                if dbg == "tbf":
                    nc.sync.dma_start(out=dbg_d[blk], in_=tbf[:])

                # ---- per-block window buffers ----
                capb = winp.tile([128, GPB, 6, F], F32, tag="capb", name="capb")
                extb = winp.tile([128, GPB, 4, F], F32, tag="extb", name="extb")
                xb = winp.tile([128, GPB, F], F32, tag="xb", name="xb")
                crb = winp.tile([128, GPB, F], F32, tag="crb", name="crb")
                idxb = winp.tile([128, GPB, F], F32, tag="idxb", name="idxb")
                shb = winp.tile([128, GPB, F], F32, tag="shb", name="shb")

                # ---- out-GEMM + spline per 128-row chunk ----
                for gi in range(GPB):
                    c = blk * GPB + gi
                    csl = slice(c * 128, (c + 1) * 128)
                    gsl = slice(gi * 128, (gi + 1) * 128)
                    nc.sync.dma_start(out=xb[:, gi, :], in_=pred[csl, :])

                    EW = spl.tile([128, F, NB], BF16, tag="EW", name="EW")
                    EH = spl.tile([128, F, NB], BF16, tag="EH", name="EH")
                    Dt = spl1.tile([128, F, NB + 1], BF16, tag="Dt", name="Dt")
                    for ty in range(3):       # 0=w 1=h 2=d
                        for pa, pair in enumerate(PAIRS):
                            ncols = DQ if ty == 2 else WQ
                            ps = psb.tile([128, 2, 512], F32, tag="po",
                                          name="po")
                            for si, q in enumerate(pair):
                                for k in range(q + 1):
                                    off = ((q - k) * QCOLS + ty * WQ)
                                    nc.tensor.matmul(
                                        ps[:, si, 0:ncols],
                                        tbf[:, k, gsl],
                                        wo_t[k][:, off:off + ncols],
                                        start=(k == 0), stop=(k == q))
                            for si, q in enumerate(pair):
                                fsl = slice(q * FQ, (q + 1) * FQ)
                                if ty == 0:
                                    nc.scalar.activation(
                                        out=EW[:, fsl, :], in_=ps[:, si, 0:ncols],
                                        func=ACTF.Exp, scale=SCALE)
                                elif ty == 1:
                                    nc.scalar.activation(
                                        out=EH[:, fsl, :], in_=ps[:, si, 0:ncols],
                                        func=ACTF.Exp, scale=SCALE)
                                else:
                                    nc.scalar.activation(
                                        out=Dt[:, fsl, :], in_=ps[:, si, 0:ncols],
                                        func=ACTF.Exp)
                    # D = ln(exp(ud) + 1) in place
                    nc.scalar.activation(
                        out=Dt[:].rearrange("p f n -> p (f n)"),
                        in_=Dt[:].rearrange("p f n -> p (f n)"),
                        func=ACTF.Ln, bias=one_t[:])
                    dD = spl.tile([128, F, NB], BF16, tag="dD", name="dD")
                    nc.gpsimd.tensor_tensor(out=dD[:], in0=Dt[:, :, 1:NB + 1],
                                            in1=Dt[:, :, 0:NB], op=OP.subtract)
                    # extracts: EW0, EH0, D0, D1
                    for j, (src, st, o) in enumerate((
                            (EW, NB, 0), (EH, NB, 0), (Dt, NB + 1, 0),
                            (Dt, NB + 1, 1))):
                        nc.scalar.activation(
                            out=extb[:, gi, j, :],
                            in_=bass.AP(tensor=src[:].tensor,
                                        offset=src[:].offset + o,
                                        ap=[src[:].ap[0], [st, F]]),
                            func=ACTF.Copy)

                    if dbg == "EW":
                        nc.sync.dma_start(out=dbg_d[c], in_=EW[:])
                    if dbg == "EH":
                        nc.sync.dma_start(out=dbg_d[c], in_=EH[:])
                    if dbg == "D":
                        nc.sync.dma_start(out=dbg_d[c], in_=Dt[:])
                    # chained cumsum of EW (fp32) for edges
                    Gg = spl.tile([128, F, NB], F32, tag="Gg", name="Gg")
                    nc.vector._custom_dve(
                        cumsum_op,
                        out=Gg[:].rearrange("p f n -> p (f n)"),
                        in0=EW[:].rearrange("p f n -> p (f n)"))
                    # per-feature smalls: Gl, S, rS, CR, xpc
                    Gl = spl1.tile([128, F], F32, tag="Gl", name="Gl")
                    tscopy(Gl[:], bass.AP(tensor=Gg[:].tensor,
                                          offset=Gg[:].offset + NB - 1,
                                          ap=[Gg[:].ap[0], [NB, F]]))
                    Sf = spl1.tile([128, F], F32, tag="Sf", name="Sf")
                    tscopy(Sf[:, 0:1], Gl[:, 0:1])
                    TT(out=Sf[:, 1:F], in0=Gl[:, 1:F], in1=Gl[:, 0:F - 1],
                       op=OP.subtract)
                    rS = spl1.tile([128, F], F32, tag="rS", name="rS")
                    nc.vector.reciprocal_approx_fast(out=rS[:], in_=Sf[:])
                    TS(out=crb[:, gi, :], in0=rS[:], scalar1=CF, scalar2=None,
                       op0=OP.mult)
                    glcr = spl1.tile([128, F], F32, tag="glcr", name="glcr")
                    # glcr[f] = Gl[f-1] * CR[f]  (current feature's CR!)
                    TT(out=glcr[:, 1:F], in0=Gl[:, 0:F - 1],
                       in1=crb[:, gi, 1:F], op=OP.mult)
                    xpc = spl1.tile([128, F], F32, tag="xpc", name="xpc")
                    tscopy(xpc[:, 0:1], xb[:, gi, 0:1])
                    TT(out=xpc[:, 1:F], in0=xb[:, gi, 1:F],
                       in1=glcr[:, 1:F], op=OP.add)

                    # edges: ENk = Gg[:, :, 0:29] * CR + k1
                    ENk = spl1.tile([128, F, NB - 1], F32, tag="ENk",
                                    name="ENk")
                    nc.gpsimd.tensor_tensor(out=ENk[:], in0=Gg[:, :, 0:NB - 1],
                                            in1=bcast(crb[:, gi, :], NB - 1),
                                            op=OP.mult)
                    k1b = bass.AP(tensor=k1_t[:].tensor, offset=k1_t[:].offset,
                                  ap=[k1_t[:].ap[0], [0, F], [1, NB - 1]])
                    nc.gpsimd.tensor_tensor(out=ENk[:], in0=ENk[:], in1=k1b,
                                            op=OP.add)
                    # u = xpc >= ENk
                    u = spl.tile([128, F, NB - 1], BF16, tag="u", name="u")
                    if DVE_BCAST:
                        TT(out=u[:], in0=bcast(xpc[:], NB - 1), in1=ENk[:],
                           op=OP.is_ge)
                    else:
                        xkb = spl1.tile([128, F, NB - 1], F32, tag="xkb",
                                        name="xkb")
                        nc.gpsimd.tensor_tensor(out=xkb[:],
                                                in0=bcast(xpc[:], NB - 1),
                                                in1=ENk[:], op=OP.subtract)
                        TS(out=u[:], in0=xkb[:], scalar1=0.0, scalar2=None,
                           op0=OP.is_ge)
                    if dbg == "Gg":
                        nc.sync.dma_start(out=dbg_d[c], in_=Gg[:])
                    if dbg == "ENk":
                        nc.sync.dma_start(out=dbg_d[c], in_=ENk[:])
                    if dbg == "xpc":
                        nc.sync.dma_start(out=dbg_d[c], in_=xpc[:])
                    if dbg == "u":
                        nc.sync.dma_start(out=dbg_d[c], in_=u[:])
                    # idx and SH reductions
                    if GP_REDUCE:
                        nc.gpsimd.tensor_reduce(out=idxb[:, gi, :], in_=u[:],
                                                axis=AX.X, op=OP.add)
                        nc.gpsimd.tensor_reduce(out=shb[:, gi, :], in_=EH[:],
                                                axis=AX.X, op=OP.add)
                    else:
                        nc.vector.tensor_reduce(out=idxb[:, gi, :], in_=u[:],
                                                axis=AX.X, op=OP.add)
                        nc.vector.tensor_reduce(out=shb[:, gi, :], in_=EH[:],
                                                axis=AX.X, op=OP.add)

                    # six masked-sum scans with stride-0 capture outputs
                    streams = (EW[:, :, 0:NB - 1], EW[:, :, 1:NB],
                               EH[:, :, 0:NB - 1], EH[:, :, 1:NB],
                               dD[:, :, 0:NB - 1], dD[:, :, 1:NB])
                    for s, tsl in enumerate(streams):
                        cap = bass.AP(
                            tensor=capb[:].tensor,
                            offset=capb[:].offset + (gi * 6 + s) * F,
                            ap=[capb[:].ap[0], [1, F], [0, NB - 1]])
                        nc.vector._custom_dve(scan_mul, out=cap,
                                              in0=u[:], in1=tsl)

                if dbg == "cap":
                    nc.sync.dma_start(out=dbg_d[blk], in_=capb[:])
                if dbg == "sm":
                    nc.sync.dma_start(out=dbg_d[blk, :, 0], in_=idxb[:])
                    nc.sync.dma_start(out=dbg_d[blk, :, 1], in_=shb[:])
                    nc.sync.dma_start(out=dbg_d[blk, :, 2], in_=crb[:])
                # ---- deferred per-feature chain, [128, GPB*F] tiles ----
                def ct(nm):
                    return chn.tile([128, GPB, F], F32, tag=nm, name=nm)

                def flat(t):
                    return t[:].rearrange("p a f -> p (a f)")

                # un-chain captures: gd[f] = cap[f] - cap[f-1] within each gi
                gd = chn.tile([128, GPB, 6, F], F32, tag="gd", name="gd")
                tscopy(gd[:, :, :, 0:1], capb[:, :, :, 0:1])
                TT(out=gd[:, :, :, 1:F], in0=capb[:, :, :, 1:F],
                   in1=capb[:, :, :, 0:F - 1], op=OP.subtract)
                g1 = gd[:, :, 0, :]
                g2 = gd[:, :, 1, :]
                g3 = gd[:, :, 2, :]
                g4 = gd[:, :, 3, :]
                g5 = gd[:, :, 4, :]
                g6 = gd[:, :, 5, :]
                ew0 = extb[:, :, 0, :]
                eh0 = extb[:, :, 1, :]
                d0e = extb[:, :, 2, :]
                d1e = extb[:, :, 3, :]

                w1 = ct("w1"); w2 = ct("w2"); w3 = ct("w3")
                w6 = ct("w6"); w7 = ct("w7"); w8 = ct("w8"); w9 = ct("w9")
                # w1 = incw = g1*CR + mb*idx
                TT(out=w1[:], in0=g1, in1=crb[:], op=OP.mult)
                STT(out=w1[:], in0=idxb[:], scalar=MIN_BIN, in1=w1[:],
                    op0=OP.mult, op1=OP.add)
                # w2 = ewi -> inw -> rw
                TT(out=w2[:], in0=g2, in1=g1, op=OP.subtract)
                nc.gpsimd.tensor_tensor(out=w2[:], in0=w2[:], in1=ew0,
                                        op=OP.add)
                TT(out=w2[:], in0=w2[:], in1=crb[:], op=OP.mult)
                TS(out=w2[:], in0=w2[:], scalar1=MIN_BIN, scalar2=None,
                   op0=OP.add)
                nc.vector.reciprocal_approx_fast(out=flat(w2), in_=flat(w2))
                # w3 = th
                TT(out=w3[:], in0=xb[:], in1=w1[:], op=OP.subtract)
                TT(out=w3[:], in0=w3[:], in1=w2[:], op=OP.mult)
                # w1 = ch2 ; shb <- inch ; w6 = inh
                nc.vector.reciprocal_approx_fast(out=flat(w1), in_=flat(shb))
                TS(out=w1[:], in0=w1[:], scalar1=CF, scalar2=None, op0=OP.mult)
                TT(out=w6[:], in0=g4, in1=g3, op=OP.subtract)
                nc.gpsimd.tensor_tensor(out=w6[:], in0=w6[:], in1=eh0,
                                        op=OP.add)
                TT(out=w6[:], in0=w6[:], in1=w1[:], op=OP.mult)
                TS(out=w6[:], in0=w6[:], scalar1=MIN_BIN, scalar2=None,
                   op0=OP.add)
                TT(out=shb[:], in0=g3, in1=w1[:], op=OP.mult)
                STT(out=shb[:], in0=idxb[:], scalar=MIN_BIN, in1=shb[:],
                    op0=OP.mult, op1=OP.add)
                # w1 = ind ; w7 = ttv ; w8 = th2
                STT(out=w1[:], in0=g5, scalar=MIN_DERIV, in1=d0e,
                    op0=OP.add, op1=OP.add)
                nc.scalar.activation(out=flat(w7), in_=flat(w3),
                                     func=ACTF.Identity, bias=one_t[:],
                                     scale=-1.0)
                nc.gpsimd.tensor_tensor(out=w7[:], in0=w3[:], in1=w7[:],
                                        op=OP.mult)
                nc.scalar.activation(out=flat(w8), in_=flat(w3),
                                     func=ACTF.Square)
                # w9 = dl
                TT(out=w9[:], in0=w6[:], in1=w2[:], op=OP.mult)
                # w3 = num (th dead)
                TT(out=w3[:], in0=w9[:], in1=w8[:], op=OP.mult)
                nc.gpsimd.tensor_tensor(out=w8[:], in0=w1[:], in1=w7[:],
                                        op=OP.mult)
                TT(out=w3[:], in0=w3[:], in1=w8[:], op=OP.add)
                TT(out=w3[:], in0=w3[:], in1=w6[:], op=OP.mult)
                # w8 = den -> rden
                STT(out=w8[:], in0=g6, scalar=MIN_DERIV, in1=d1e,
                    op0=OP.add, op1=OP.add)
                TT(out=w8[:], in0=w8[:], in1=w1[:], op=OP.add)
                STT(out=w8[:], in0=w9[:], scalar=-2.0, in1=w8[:],
                    op0=OP.mult, op1=OP.add)
                TT(out=w8[:], in0=w8[:], in1=w7[:], op=OP.mult)
                TT(out=w8[:], in0=w8[:], in1=w9[:], op=OP.add)
                nc.vector.reciprocal_approx_fast(out=flat(w8), in_=flat(w8))
                cdf = w3
                TT(out=cdf[:], in0=cdf[:], in1=w8[:], op=OP.mult)
                TT(out=cdf[:], in0=cdf[:], in1=shb[:], op=OP.add)
                # product over 64 features
                TT(out=cdf[:, :, 0:32], in0=cdf[:, :, 0:32],
                   in1=cdf[:, :, 32:64], op=OP.mult)
                TT(out=cdf[:, :, 0:16], in0=cdf[:, :, 0:16],
                   in1=cdf[:, :, 16:32], op=OP.mult)
                TT(out=cdf[:, :, 0:8], in0=cdf[:, :, 0:8],
                   in1=cdf[:, :, 8:16], op=OP.mult)
                TT(out=cdf[:, :, 0:4], in0=cdf[:, :, 0:4],
                   in1=cdf[:, :, 4:8], op=OP.mult)
                TT(out=cdf[:, :, 0:2], in0=cdf[:, :, 0:2],
                   in1=cdf[:, :, 2:4], op=OP.mult)
                TT(out=prodb[:, blk * GPB:(blk + 1) * GPB],
                   in0=cdf[:, :, 0:1].rearrange("p a f -> p (a f)"),
                   in1=cdf[:, :, 1:2].rearrange("p a f -> p (a f)"),
                   op=OP.mult)

            nc.sync.dma_start(out=out_d.rearrange("(c p) -> p c", p=128),
                              in_=prodb[:])

    nc.compile()
    return nc


def _to_bf16(a):
    import ml_dtypes
    return np.ascontiguousarray(np.asarray(a, np.float32)).astype(
        ml_dtypes.bfloat16)


def _prep_shared(W_in, b_in, Wc_in, bc_in, Wb1, bb1, Wb2, bb2, Wcb, bcb,
                 W_out, b_out):
    m_in, m_hh, m_out, d_h = _masks()
    assert not np.any(np.asarray(b_out)), "nonzero b_out not supported"
    perm = np.argsort(d_h, kind="stable")

    W_in = np.asarray(W_in, np.float32) * m_in
    W_in = W_in[:, perm]
    Wc_in = np.asarray(Wc_in, np.float32)[:, perm]
    Wcb = np.asarray(Wcb, np.float32)[:, :, perm]
    Wb1p = (np.asarray(Wb1, np.float32) * m_hh[None])[:, perm][:, :, perm]
    Wb2p = (np.asarray(Wb2, np.float32) * m_hh[None])[:, perm][:, :, perm]
    W_outp = (np.asarray(W_out, np.float32) * m_out)[perm, :]
    b1 = (np.asarray(b_in, np.float32) + np.asarray(bc_in, np.float32))[perm]
    bb1p = np.asarray(bb1, np.float32)[:, perm]
    bb2p = np.asarray(bb2, np.float32)[:, perm]
    bcbp = np.asarray(bcb, np.float32)[:, perm]

    wc4 = np.concatenate([Wc_in, Wcb[0], Wcb[1], Wcb[2]], axis=1)

    # packed W_out blobs: chunk k holds quarters q>=k as [w(480)|h(480)|d(496)]
    wo = []
    for k in range(KH):
        cols = []
        for q in range(k, NQ):
            for ty in range(3):
                for f in range(FQ * q, FQ * (q + 1)):
                    if ty == 0:
                        cols.extend(range(f * MULT, f * MULT + NB))
                    elif ty == 1:
                        cols.extend(range(f * MULT + NB, f * MULT + 2 * NB))
                    else:
                        cols.extend(range(f * MULT + 2 * NB, (f + 1) * MULT))
        blob = W_outp[k * 128:(k + 1) * 128][:, cols]
        assert blob.shape == (128, (NQ - k) * QCOLS)
        wo.append(_to_bf16(blob))

    shared = {
        "win": _to_bf16(W_in),
        "wc4": _to_bf16(wc4),
        "wb1": _to_bf16(Wb1p),
        "wb2": _to_bf16(Wb2p),
        "b1": np.ascontiguousarray(b1),
        "bb1": np.ascontiguousarray(bb1p),
        "bb2": np.ascontiguousarray(bb2p),
        "bcb": np.ascontiguousarray(bcbp),
        "ident": np.eye(128, dtype=np.float32),
        "k1c": (MIN_BIN * np.arange(1, NB)).astype(np.float32),
    }
    for k in range(KH):
        shared[f"wo{k}"] = wo[k]
    return shared


def kernel(predicates, contexts, W_in, b_in, Wc_in, bc_in, Wb1, bb1, Wb2, bb2,
           Wcb, bcb, W_out, b_out):
    global LAST_RESULTS
    predicates = np.asarray(predicates, dtype=np.float32)
    contexts = np.asarray(contexts, dtype=np.float32)
    bc = predicates.shape[0] // NCORES
    key = (bc, GP_REDUCE, DVE_BCAST, DBG)
    if key not in _CACHE:
        _CACHE[key] = _build(bc, dbg=DBG)
    nc = _CACHE[key]
    shared = _prep_shared(W_in, b_in, Wc_in, bc_in, Wb1, bb1, Wb2, bb2,
                          Wcb, bcb, W_out, b_out)
    in_maps = []
    for cid in range(NCORES):
        sl = slice(cid * bc, (cid + 1) * bc)
        m = dict(shared)
        m["pred"] = np.ascontiguousarray(predicates[sl])
        m["ctx"] = np.ascontiguousarray(contexts[sl])
        in_maps.append(m)
    res = run_bass_kernel_spmd(nc, in_maps, core_ids=list(range(NCORES)),
                               trace=TRACE)
    LAST_RESULTS = res
    return np.concatenate([res.results[i]["out"] for i in range(NCORES)])
